# revision 1
# baseline (speedup 1.0000x reference)
"""Trainium2 Bass kernel for the GNN ExplainModule (masked adjacency).

Strategy (8 NeuronCores, row-sharded output):
  - Each core owns 1250 rows of the [10000, 10000] output, processed in
    row-blocks of 128.
  - Host routes each edge's two contributions ((r,c) and (c,r), weight
    0.5*gate) to the owning core/block, sorted by destination; indices
    only — all FP math runs on device.
  - Device tables via PE: A = (embed @ W1a + 1 x c_vec) * |W2|,
    B = (embed @ W1b) * |W2|  (hidden units permuted so W2 >= 0 first;
    signs re-applied as pos-reduce minus neg-reduce).
  - Per contribution: dma_gather A[row], B[col] and the 64-wide adj
    segment holding (r, c); compute gate = sigmoid(logit(noise) + mlp);
    payload = onehot64(c % 64) * adj_seg * (0.5 * gate * valid);
    dma_scatter_add payload into the output (CCE add; duplicate dests
    accumulate natively; output buffers arrive pre-zeroed via PJRT
    donation so untouched cells stay 0).
"""

import sys

import numpy as np

for _p in ("/opt/trn_rl_repo",):
    if _p not in sys.path:
        sys.path.insert(0, _p)

N = 10000
D = 64
NCORES = 8
RPC = N // NCORES  # rows per core
BLK = 128  # rows per block
SEG = -(-N // 64)  # 64-wide segments per row (157)
SEGX = SEG + 1  # +1 pad segment per row (scatter pad target)
PITCH = SEGX * 64  # padded row pitch
SUB = 1024  # tokens per custom-DMA op


def _blocks():
    out = []
    r = 0
    while r < RPC:
        h = min(BLK, RPC - r)
        out.append((r, h))
        r += h
    return out


def _prep_host(row, col, noise):
    """Route contributions to (core, block); build packed token arrays."""
    row = np.asarray(row).astype(np.int64).ravel()
    col = np.asarray(col).astype(np.int64).ravel()
    noise = np.asarray(noise).astype(np.float32).ravel()

    dr = np.concatenate([row, col])  # dest row
    dc = np.concatenate([col, row])  # dest col
    ea = np.concatenate([row, row])  # A-table index
    eb = np.concatenate([col, col])  # B-table index
    en = np.concatenate([noise, noise])
    core = dr // RPC

    blocks = _blocks()
    nblk = len(blocks)
    # per core, per block, per wave: token arrays. A scatter instruction must
    # not carry two tokens targeting the same 64-wide segment row (the HW CCE
    # adds race within one instruction); the w-th token of each segment group
    # goes to wave w, and waves scatter in separate, serialized instructions.
    toks = [[None] * nblk for _ in range(NCORES)]
    n_waves = 1
    for k in range(NCORES):
        m = core == k
        rl = dr[m] - k * RPC
        d = rl * N + dc[m]
        o = np.argsort(d, kind="stable")
        rl, dcc, a, b, nz = rl[o], dc[m][o], ea[m][o], eb[m][o], en[m][o]
        blk_id = rl // BLK
        for bi, (r0, h) in enumerate(blocks):
            sel = blk_id == bi
            si = (rl[sel] - r0) * SEGX + dcc[sel] // 64
            # occurrence rank of each token within its segment group (tokens
            # are sorted by dest, so equal si values are adjacent)
            uq, inv, cnt = np.unique(si, return_inverse=True, return_counts=True)
            starts = np.zeros(len(uq) + 1, np.int64)
            np.cumsum(cnt, out=starts[1:])
            rank = np.arange(len(si)) - starts[inv]
            n_waves = max(n_waves, int(cnt.max()) if len(cnt) else 1)
            toks[k][bi] = (
                a[sel],
                b[sel],
                nz[sel],
                si,
                (dcc[sel] % 64).astype(np.float32),
                rank,
            )

    # SPMD-static chunk sizes per (block, wave)
    chunk_list = []  # (block_idx, row0, blk_h, t, off16, off128)
    key_sizes = {}  # (bi, w) -> padded size
    off16 = off128 = 0
    for bi, (r0, h) in enumerate(blocks):
        for w in range(n_waves):
            t_bw = max(
                int((toks[k][bi][5] == w).sum()) for k in range(NCORES)
            )
            if w == 0:
                t_bw = max(t_bw, 1)
            if t_bw == 0:
                continue
            t_bw = -(-t_bw // 128) * 128
            key_sizes[(bi, w)] = t_bw
            done = 0
            while done < t_bw:
                t = min(SUB, t_bw - done)
                chunk_list.append((bi, r0, h, t, off16, off128))
                off16 += t // 16
                off128 += t // 128
                done += t
    total16, total128 = off16, off128

    pad_si = SEGX - 1  # row 0's pad segment; never holds real data

    per_core = []
    for k in range(NCORES):
        ga16 = np.zeros((128, total16), np.int16)
        gb16 = np.zeros((128, total16), np.int16)
        si16 = np.full((128, total16), 0, np.int16)
        nzf = np.full((128, total128), 0.5, np.float32)
        cmf = np.zeros((128, total128), np.float32)
        vmf = np.zeros((128, total128), np.float32)
        ci = 0
        for bi, (r0, h) in enumerate(blocks):
            a0, b0, nz0, si0, cm0, rank0 = toks[k][bi]
            for w in range(n_waves):
                if (bi, w) not in key_sizes:
                    continue
                t_bw = key_sizes[(bi, w)]
                sel = rank0 == w
                n = int(sel.sum())
                pad = t_bw - n
                a = np.concatenate([a0[sel], np.zeros(pad, np.int64)])
                b = np.concatenate([b0[sel], np.zeros(pad, np.int64)])
                nz = np.concatenate([nz0[sel], np.full(pad, 0.5, np.float32)])
                si = np.concatenate([si0[sel], np.full(pad, pad_si, np.int64)])
                cm = np.concatenate([cm0[sel], np.zeros(pad, np.float32)])
                vm = np.concatenate(
                    [np.ones(n, np.float32), np.zeros(pad, np.float32)]
                )
                done = 0
                while done < t_bw:
                    bi2, _r0, _h, t, o16, o128 = chunk_list[ci]
                    assert bi2 == bi and done + t <= t_bw
                    sl = slice(done, done + t)

                    def wrap16(x):
                        return np.tile(
                            np.ascontiguousarray(x[sl].reshape(-1, 16).T),
                            (8, 1),
                        )

                    def wrap128(x):
                        return np.ascontiguousarray(x[sl].reshape(-1, 128).T)

                    ga16[:, o16 : o16 + t // 16] = wrap16(a).astype(np.int16)
                    gb16[:, o16 : o16 + t // 16] = wrap16(b).astype(np.int16)
                    si16[:, o16 : o16 + t // 16] = wrap16(si).astype(np.int16)
                    nzf[:, o128 : o128 + t // 128] = wrap128(nz)
                    cmf[:, o128 : o128 + t // 128] = wrap128(cm)
                    vmf[:, o128 : o128 + t // 128] = wrap128(vm)
                    done += t
                    ci += 1
        assert ci == len(chunk_list)
        per_core.append(
            dict(ga16=ga16, gb16=gb16, si16=si16, nz=nzf, cm=cmf, vm=vmf)
        )
    return per_core, chunk_list, total16, total128


def _build_program(chunk_list, total16, total128, node_idx, b2f, pos_cnt):
    import concourse.bacc as bacc
    import concourse.bass as bass
    import concourse.mybir as mybir
    import concourse.tile as tile
    from concourse.masks import make_identity

    f32 = mybir.dt.float32
    i16 = mybir.dt.int16
    add = mybir.AluOpType.add
    mult = mybir.AluOpType.mult
    subtract = mybir.AluOpType.subtract
    is_equal = mybir.AluOpType.is_equal
    AF = mybir.ActivationFunctionType

    nc = bacc.Bacc()

    blocks = _blocks()
    out_rows = sum(BLK for _ in blocks)  # padded block heights (128 each)

    embp = nc.declare_dram_parameter("embed", [N, D], f32, isOutput=False)
    w1p = nc.declare_dram_parameter("w1", [3 * D, D], f32, isOutput=False)
    b1p = nc.declare_dram_parameter("b1r", [1, D], f32, isOutput=False)
    w2p = nc.declare_dram_parameter("w2b", [128, D], f32, isOutput=False)
    iop = nc.declare_dram_parameter("iota64", [128, D], f32, isOutput=False)
    adjp = nc.declare_dram_parameter("adjp", [out_rows, PITCH], f32, isOutput=False)
    gap = nc.declare_dram_parameter("ga16", [128, total16], i16, isOutput=False)
    gbp = nc.declare_dram_parameter("gb16", [128, total16], i16, isOutput=False)
    sip = nc.declare_dram_parameter("si16", [128, total16], i16, isOutput=False)
    nzp = nc.declare_dram_parameter("nz", [128, total128], f32, isOutput=False)
    cmp_ = nc.declare_dram_parameter("cm", [128, total128], f32, isOutput=False)
    vmp = nc.declare_dram_parameter("vm", [128, total128], f32, isOutput=False)
    outp = nc.declare_dram_parameter("out", [out_rows, PITCH], f32, isOutput=True)

    a_dram = nc.dram_tensor("a_table", [N, D], f32)
    b_dram = nc.dram_tensor("b_table", [N, D], f32)

    NBLKA = -(-N // 128)

    with tile.TileContext(nc) as tc:
        with (
            tc.tile_pool(name="const", bufs=1) as cp,
            tc.tile_pool(name="stagea", bufs=3) as sp,
            tc.tile_pool(name="work", bufs=2) as wp,
            tc.tile_pool(name="psum", bufs=2, space="PSUM") as pp,
        ):
            identity = cp.tile([128, 128], f32)
            make_identity(nc, identity[:])
            w1a = cp.tile([D, D], f32)
            nc.sync.dma_start(out=w1a[:], in_=w1p[0:D, :])
            w1b = cp.tile([D, D], f32)
            nc.sync.dma_start(out=w1b[:], in_=w1p[D : 2 * D, :])
            w1c = cp.tile([D, D], f32)
            nc.sync.dma_start(out=w1c[:], in_=w1p[2 * D : 3 * D, :])
            b1t = cp.tile([1, D], f32)
            nc.sync.dma_start(out=b1t[:], in_=b1p[:, :])
            w2t = cp.tile([128, D], f32)
            nc.sync.dma_start(out=w2t[:], in_=w2p[:, :])
            iot = cp.tile([128, D], f32)
            nc.sync.dma_start(out=iot[:], in_=iop[:, :])
            ones = cp.tile([1, 128], f32)
            nc.vector.memset(ones[:], 1.0)
            e5 = cp.tile([D, 1], f32)
            nc.sync.dma_start(
                out=e5[:], in_=embp[node_idx : node_idx + 1, :].rearrange("o d -> d o")
            )

            # c_vec = embed[node_idx] @ W1c + b1  -> [1, D]
            cps = pp.tile([1, D], f32, tag="cps")
            nc.tensor.matmul(cps[:], lhsT=e5[:], rhs=w1c[:], start=True, stop=True)
            crow = cp.tile([1, D], f32)
            nc.vector.tensor_tensor(out=crow[:], in0=cps[:], in1=b1t[:], op=add)

            # Stage A: A = (embed @ W1a + 1 x crow) * |W2| ; B = (embed @ W1b) * |W2|
            for blk in range(NBLKA):
                r0 = blk * 128
                p = min(128, N - r0)
                et = sp.tile([128, D], f32, tag="et")
                nc.sync.dma_start(out=et[:p, :], in_=embp[r0 : r0 + p, :])
                tps = pp.tile([D, 128], f32, tag="tps")
                nc.tensor.transpose(tps[:, :p], et[:p, :], identity[:p, :p])
                tsb = sp.tile([D, 128], f32, tag="tsb")
                nc.scalar.copy(out=tsb[:, :p], in_=tps[:, :p])
                pa_ = pp.tile([128, D], f32, tag="pa")
                nc.tensor.matmul(
                    pa_[:p, :], lhsT=tsb[:, :p], rhs=w1a[:], start=True, stop=False
                )
                nc.tensor.matmul(
                    pa_[:p, :], lhsT=ones[:, :p], rhs=crow[:], start=False, stop=True
                )
                asb = sp.tile([128, D], f32, tag="asb")
                nc.vector.tensor_tensor(
                    out=asb[:p, :], in0=pa_[:p, :], in1=w2t[:p, :], op=mult
                )
                nc.sync.dma_start(out=a_dram[r0 : r0 + p, :], in_=asb[:p, :])
                pb_ = pp.tile([128, D], f32, tag="pb")
                nc.tensor.matmul(
                    pb_[:p, :], lhsT=tsb[:, :p], rhs=w1b[:], start=True, stop=True
                )
                bsb = sp.tile([128, D], f32, tag="bsb")
                nc.vector.tensor_tensor(
                    out=bsb[:p, :], in0=pb_[:p, :], in1=w2t[:p, :], op=mult
                )
                nc.sync.dma_start(out=b_dram[r0 : r0 + p, :], in_=bsb[:p, :])

            # contribution chunks
            for bi, r0b, h, t, o16, o128 in chunk_list:
                S = t // 128
                S16 = t // 16
                gai = wp.tile([128, S16], i16, tag="gai")
                nc.sync.dma_start(out=gai[:], in_=gap[:, o16 : o16 + S16])
                gbi = wp.tile([128, S16], i16, tag="gbi")
                nc.sync.dma_start(out=gbi[:], in_=gbp[:, o16 : o16 + S16])
                sii = wp.tile([128, S16], i16, tag="sii")
                nc.sync.dma_start(out=sii[:], in_=sip[:, o16 : o16 + S16])
                nz = wp.tile([128, S], f32, tag="nz")
                nc.sync.dma_start(out=nz[:], in_=nzp[:, o128 : o128 + S])
                cm = wp.tile([128, S], f32, tag="cm")
                nc.sync.dma_start(out=cm[:], in_=cmp_[:, o128 : o128 + S])
                vm = wp.tile([128, S], f32, tag="vm")
                nc.sync.dma_start(out=vm[:], in_=vmp[:, o128 : o128 + S])

                ga = wp.tile([128, S * D], f32, tag="ga")
                nc.gpsimd.dma_gather(
                    out_ap=ga[:].rearrange("p (s d) -> p s d", d=D),
                    in_ap=a_dram[:, :],
                    idxs_ap=gai[:],
                    num_idxs=t,
                    num_idxs_reg=t,
                    elem_size=D,
                )
                gb = wp.tile([128, S * D], f32, tag="gb")
                nc.gpsimd.dma_gather(
                    out_ap=gb[:].rearrange("p (s d) -> p s d", d=D),
                    in_ap=b_dram[:, :],
                    idxs_ap=gbi[:],
                    num_idxs=t,
                    num_idxs_reg=t,
                    elem_size=D,
                )
                adjseg = wp.tile([128, S * D], f32, tag="adjseg")
                adj_view = adjp[r0b : r0b + BLK, :].rearrange(
                    "p (s w) -> (p s) w", w=64
                )
                nc.gpsimd.dma_gather(
                    out_ap=adjseg[:].rearrange("p (s d) -> p s d", d=D),
                    in_ap=adj_view,
                    idxs_ap=sii[:],
                    num_idxs=t,
                    num_idxs_reg=t,
                    elem_size=D,
                )

                # MLP: pre = ga + gb ; q = relu(pre) ; s = sum_pos - sum_neg
                nc.vector.tensor_tensor(out=ga[:], in0=ga[:], in1=gb[:], op=add)
                nc.scalar.activation(out=ga[:], in_=ga[:], func=AF.Relu)
                q3 = ga[:].rearrange("p (s d) -> p s d", d=D)
                s = wp.tile([128, S], f32, tag="s")
                if pos_cnt == D:
                    nc.vector.tensor_reduce(
                        out=s[:], in_=q3, axis=mybir.AxisListType.X, op=add
                    )
                elif pos_cnt == 0:
                    nc.vector.tensor_reduce(
                        out=s[:], in_=q3, axis=mybir.AxisListType.X, op=add,
                        negate=True,
                    )
                else:
                    nc.vector.tensor_reduce(
                        out=s[:], in_=q3[:, :, :pos_cnt],
                        axis=mybir.AxisListType.X, op=add,
                    )
                    sn = wp.tile([128, S], f32, tag="sn")
                    nc.vector.tensor_reduce(
                        out=sn[:], in_=q3[:, :, pos_cnt:],
                        axis=mybir.AxisListType.X, op=add,
                    )
                    nc.vector.tensor_tensor(
                        out=s[:], in0=s[:], in1=sn[:], op=subtract
                    )

                # gate = sigmoid(ln(nz) - ln(1-nz) + s + b2)
                om = wp.tile([128, S], f32, tag="om")
                nc.vector.tensor_scalar(
                    out=om[:], in0=nz[:], scalar1=-1.0, scalar2=1.0,
                    op0=mult, op1=add,
                )
                ln1 = wp.tile([128, S], f32, tag="ln1")
                nc.scalar.activation(out=ln1[:], in_=nz[:], func=AF.Ln)
                ln2 = wp.tile([128, S], f32, tag="ln2")
                nc.scalar.activation(out=ln2[:], in_=om[:], func=AF.Ln)
                z = wp.tile([128, S], f32, tag="z")
                nc.vector.scalar_tensor_tensor(
                    out=z[:], in0=ln1[:], scalar=b2f, in1=ln2[:],
                    op0=add, op1=subtract,
                )
                nc.vector.tensor_tensor(out=z[:], in0=z[:], in1=s[:], op=add)
                g = wp.tile([128, S], f32, tag="g")
                nc.scalar.activation(out=g[:], in_=z[:], func=AF.Sigmoid)
                gm = wp.tile([128, S], f32, tag="gm")
                nc.vector.scalar_tensor_tensor(
                    out=gm[:], in0=g[:], scalar=0.5, in1=vm[:],
                    op0=mult, op1=mult,
                )

                # payload = onehot(cm) * adjseg * gm
                oh = wp.tile([128, S * D], f32, tag="oh")
                oh3 = oh[:].rearrange("p (s d) -> p s d", d=D)
                io_b = iot[:].rearrange("p (o d) -> p o d", o=1).to_broadcast(
                    [128, S, D]
                )
                cm_b = cm[:].rearrange("p (s o) -> p s o", o=1).to_broadcast(
                    [128, S, D]
                )
                nc.vector.tensor_tensor(out=oh3, in0=io_b, in1=cm_b, op=is_equal)
                nc.vector.tensor_tensor(out=oh[:], in0=oh[:], in1=adjseg[:], op=mult)
                gm_b = gm[:].rearrange("p (s o) -> p s o", o=1).to_broadcast(
                    [128, S, D]
                )
                nc.vector.tensor_tensor(out=oh3, in0=oh3, in1=gm_b, op=mult)

                out_view = outp[r0b : r0b + BLK, :].rearrange(
                    "p (s w) -> (p s) w", w=64
                )
                nc.gpsimd.dma_scatter_add(
                    out_ap=out_view,
                    in_ap=oh[:].rearrange("p (s d) -> p s d", d=D),
                    idxs_ap=sii[:],
                    num_idxs=t,
                    num_idxs_reg=t,
                    elem_size=D,
                )

    nc.compile()
    return nc


def kernel(embed, row, col, adj, noise, W1, b1, W2, b2, node_idx):
    from concourse.bass_utils import run_bass_kernel_spmd

    embed = np.ascontiguousarray(np.asarray(embed), dtype=np.float32)
    adj = np.ascontiguousarray(np.asarray(adj), dtype=np.float32)
    W1 = np.ascontiguousarray(np.asarray(W1), dtype=np.float32)
    b1 = np.ascontiguousarray(np.asarray(b1), dtype=np.float32).ravel()
    W2 = np.ascontiguousarray(np.asarray(W2), dtype=np.float32)
    b2f = float(np.asarray(b2, dtype=np.float32).ravel()[0])
    nidx = int(np.asarray(node_idx))

    # permute hidden units: W2 >= 0 first; fold |W2| on device
    w2v = W2.reshape(-1).astype(np.float32)
    order = np.argsort(w2v < 0, kind="stable")
    pos_cnt = int((w2v >= 0).sum())
    W1p = np.ascontiguousarray(W1[:, order])
    b1p = np.ascontiguousarray(b1[order]).reshape(1, D)
    w2b = np.ascontiguousarray(
        np.tile(np.abs(w2v[order]).reshape(1, D), (128, 1))
    )
    iota64 = np.ascontiguousarray(
        np.tile(np.arange(D, dtype=np.float32).reshape(1, D), (128, 1))
    )

    per_core, chunk_list, total16, total128 = _prep_host(row, col, noise)
    nc = _build_program(chunk_list, total16, total128, nidx, b2f, pos_cnt)

    blocks = _blocks()
    out_rows = BLK * len(blocks)
    in_maps = []
    for k in range(NCORES):
        adjpad = np.zeros((out_rows, PITCH), np.float32)
        sl = adj[k * RPC : (k + 1) * RPC]
        adjpad[: sl.shape[0], :N] = sl
        m = dict(per_core[k])
        m.update(
            embed=embed, w1=W1p, b1r=b1p, w2b=w2b, iota64=iota64, adjp=adjpad
        )
        in_maps.append(m)

    res = run_bass_kernel_spmd(nc, in_maps, list(range(NCORES)))
    kernel.last_exec_time_ns = res.exec_time_ns
    kernel.last_result = res
    pieces = []
    for k in range(NCORES):
        o = res.results[k]["out"]
        # blocks are stacked at BLK spacing; real rows of block bi: r0..r0+h
        for bi, (r0, h) in enumerate(blocks):
            pieces.append(o[bi * BLK : bi * BLK + h, :N])
    out = np.concatenate(pieces, axis=0)
    return out


kernel.last_exec_time_ns = None



# revision 2
# speedup vs baseline: 1.2099x; 1.2099x over previous
"""Trainium2 Bass kernel v2 for the GNN ExplainModule (masked adjacency).

Per core (8 cores, 1250 output rows each, 10 blocks of 128 rows):
  Phase A: tables A=(embed@W1a+cvec)|w2|, B=(embed@W1b)|w2| (hidden permuted
           pos-w2-first) -> DRAM CT2 [20000, 64] f32 (rows 0-9999=B, rest=A).
  Phase B1 (per block): SWDGE dma_gather of each token's random-endpoint
           table row into the wrap grid [128, NIeff, 64]; add the dest-row
           local table vector (per-partition broadcast); relu; pos/neg
           free-dim reduce -> per-token logit; sigmoid tail in wrap layout
           -> bf16 gate grid; duplicate-cell strips folded in with plain
           slice adds (pad slots have noise=1e-30 -> gate==0).
  Phase B2 (per block x 5 col-chunks): gpsimd.local_scatter builds the dense
           [128, 2000] bf16 mask chunk; multiply with the bf16 adj chunk;
           DMA to the bf16 output. Host upcasts and reassembles.
"""

import sys

import numpy as np

for _p in ("/opt/trn_rl_repo",):
    if _p not in sys.path:
        sys.path.insert(0, _p)

_PREP_SRC = r'''
"""Host-side routing/packing for the grid-based dense-apply kernel (v2).

Each edge (r, c) yields two tokens (contributions of 0.5*gate_e):
  type-1 -> cell (r, c): local table = A[dest row r], random = B[dest col c]
  type-2 -> cell (c, r): local table = B[dest row c], random = A[dest col r]

Combined random-gather table CT2 rows: 0..9999 = B, 10000..19999 = A.
  token gather idx = dc + 10000*type   (type in {0,1})

Grid: per (core, block) a [128, NIeff] slot grid; partition = dest row % 128;
flat (SWDGE-wrap) order j*128 + p. Columns, in order:
  A-section (A-local tiled add):  [T1 | P0 | Q(j,tq=0,tp) ...]
  B-section (B-local tiled add):  [T2 | P1 | Q(j,tq=1,tp) ...]
T1/T2: plain tokens. P0/P1: duplicate-cell primaries (type 0/1). Q(j,tq,tp):
j-th extra token (type tq) of a dup cell whose primary has type tp; device
adds Q strip into P strip (plain slice add) after gates are computed. Pad
slots get noise=PAD_NOISE so their gate underflows to 0 (no masks needed).
Primaries and T1/T2 slots carry the local_scatter column index; partners and
pads carry -1.
"""

import numpy as np

N = 10000
E = 320000
D = 64
NC = 8
RPC = N // NC  # 1250
BLK = 128
NBLK = -(-RPC // BLK)  # 10
CCH = 2000
NCHUNK = 5
PAD_NOISE = np.float32(1e-30)


def prep(row, col, noise):
    row = np.asarray(row).astype(np.int64).ravel()
    col = np.asarray(col).astype(np.int64).ravel()
    noise = np.asarray(noise).astype(np.float32).ravel()

    dr = np.concatenate([row, col])
    dc = np.concatenate([col, row])
    typ = np.concatenate([np.zeros(E, np.int64), np.ones(E, np.int64)])
    nz = np.concatenate([noise, noise])
    core = dr // RPC

    # ---- pass 1: group tokens, find dup chains, collect static sizes ------
    # cells[k][(bi,p)] = dict(t1=[(dc,ty,nz)...], t2=[...], chains=[[tok...]])
    percore = []
    NI1 = np.zeros(NBLK, np.int64)
    NI2 = np.zeros(NBLK, np.int64)
    WP0 = np.zeros(NBLK, np.int64)
    WP1 = np.zeros(NBLK, np.int64)
    WQ = [{} for _ in range(NBLK)]  # (j, tq, tp) -> width

    for k in range(NC):
        m = core == k
        rl = (dr[m] - k * RPC).astype(np.int64)
        dck = dc[m]
        tyk = typ[m]
        nzk = nz[m]
        bi = rl // BLK
        p = rl % BLK

        cell = rl * N + dck
        order = np.argsort(cell, kind="stable")
        cs = cell[order]
        starts = np.flatnonzero(np.concatenate(([True], cs[1:] != cs[:-1])))
        counts = np.diff(np.concatenate((starts, [len(cs)])))

        rows = {}
        for si, cnt in zip(starts, counts):
            idxs = order[si : si + cnt]
            i0 = idxs[0]
            key = (int(bi[i0]), int(p[i0]))
            r = rows.setdefault(key, dict(t1=[], t2=[], chains=[]))
            toks = [(int(dck[i]), int(tyk[i]), float(nzk[i])) for i in idxs]
            if cnt == 1:
                (r["t1"] if toks[0][1] == 0 else r["t2"]).append(toks[0])
            else:
                r["chains"].append(toks)
        percore.append(rows)

        for (b, _pp), r in rows.items():
            NI1[b] = max(NI1[b], len(r["t1"]))
            NI2[b] = max(NI2[b], len(r["t2"]))
            np0 = sum(1 for ch in r["chains"] if ch[0][1] == 0)
            np1 = len(r["chains"]) - np0
            WP0[b] = max(WP0[b], np0)
            WP1[b] = max(WP1[b], np1)
            qcnt = {}
            i0 = i1 = 0
            for ch in r["chains"]:
                tp = ch[0][1]
                for j, tok in enumerate(ch[1:], start=1):
                    kq = (j, tok[1], tp)
                    qcnt[kq] = qcnt.get(kq, 0) + 1
            for kq, w in qcnt.items():
                WQ[b][kq] = max(WQ[b].get(kq, 0), w)

    # ---- static grid layout per block -------------------------------------
    layout = []  # per block: dict with region offsets
    for b in range(NBLK):
        off = 0
        t1_off = off
        off += int(NI1[b])
        p0_off = off
        off += int(WP0[b])
        qA = []  # (j,tq=0,tp) strips, A-section
        for kq in sorted(kk for kk in WQ[b] if kk[1] == 0):
            qA.append((kq, off, WQ[b][kq]))
            off += WQ[b][kq]
        na = off
        t2_off = off
        off += int(NI2[b])
        p1_off = off
        off += int(WP1[b])
        qB = []
        for kq in sorted(kk for kk in WQ[b] if kk[1] == 1):
            qB.append((kq, off, WQ[b][kq]))
            off += WQ[b][kq]
        nieff = off
        if nieff % 2:
            nieff += 1
        # dup passes: for each Q strip: dst = P strip of tp
        passes = []
        for (j, tq, tp), qoff, w in qA + qB:
            passes.append(((p0_off if tp == 0 else p1_off), qoff, w))
        layout.append(
            dict(
                t1=t1_off, p0=p0_off, na=na, t2=t2_off, p1=p1_off,
                nieff=nieff, passes=passes,
                qA=qA, qB=qB,
            )
        )
    grid_off = np.zeros(NBLK + 1, np.int64)
    for b in range(NBLK):
        grid_off[b + 1] = grid_off[b] + layout[b]["nieff"]
    TOT_NI = int(grid_off[NBLK])
    TOT_T = TOT_NI * 128

    # ---- pass 2: fill per-core slot arrays --------------------------------
    cores = []
    for k in range(NC):
        rows = percore[k]
        gidx = np.zeros((128, TOT_NI), np.int64)  # gather row in CT2
        nzg = np.full((128, TOT_NI), PAD_NOISE, np.float32)
        lscol = np.full((128, TOT_NI), -1, np.int64)  # dest col or -1
        for (b, pp), r in rows.items():
            lay = layout[b]
            g0 = grid_off[b]

            def put(slot, tok, scatter):
                dcv, ty, nzv = tok
                gidx[pp, g0 + slot] = dcv + 10000 * ty
                nzg[pp, g0 + slot] = nzv
                if scatter:
                    lscol[pp, g0 + slot] = dcv

            for i, tok in enumerate(r["t1"]):
                put(lay["t1"] + i, tok, True)
            for i, tok in enumerate(r["t2"]):
                put(lay["t2"] + i, tok, True)
            ip0 = ip1 = 0
            qfill = {}
            for ch in r["chains"]:
                tp = ch[0][1]
                if tp == 0:
                    islot = ip0
                    ip0 += 1
                    put(lay["p0"] + islot, ch[0], True)
                else:
                    islot = ip1
                    ip1 += 1
                    put(lay["p1"] + islot, ch[0], True)
                for j, tok in enumerate(ch[1:], start=1):
                    kq = (j, tok[1], tp)
                    qoff = None
                    for kk, qo, w in lay["qA"] + lay["qB"]:
                        if kk == kq:
                            qoff = qo
                            break
                    # slot within strip must MATCH the primary's islot so the
                    # plain slice add P[:, 0:w] += Q[:, 0:w] lands correctly
                    put(qoff + islot, tok, False)
                    qfill.setdefault((pp, kq), []).append(islot)

        # sanity: within a strip, each (partition, islot) used at most once
        cores.append(dict(gidx=gidx, nzg=nzg, lscol=lscol))

    static = dict(
        layout=layout, grid_off=grid_off, TOT_NI=TOT_NI, TOT_T=TOT_T,
        NI1=NI1, NI2=NI2, WP0=WP0, WP1=WP1,
    )
    return cores, static


def device_arrays(cores, static):
    """Pack per-core arrays into the device input formats."""
    TOT_NI = static["TOT_NI"]
    TOT_T = static["TOT_T"]
    out = []
    for c in cores:
        gidx = c["gidx"]  # [128, TOT_NI] row in CT2
        # flat order f = j*128 + p  ->  wrapped [128, TOT_T//16] int16:
        flat = np.empty(TOT_T, np.int64)
        flat.reshape(TOT_NI, 128)[:] = gidx.T  # flat[j*128+p] = gidx[p, j]
        w = np.empty((128, TOT_T // 16), np.int16)
        s = np.arange(TOT_T // 16)
        for p in range(128):
            w[p, :] = flat[s * 16 + p % 16]
        nzg = c["nzg"].astype(np.float32)
        lsx = np.full((128, NCHUNK * TOT_NI), -1, np.int16)
        for ch in range(NCHUNK):
            sel = (c["lscol"] >= ch * CCH) & (c["lscol"] < (ch + 1) * CCH)
            v = np.where(sel, c["lscol"] - ch * CCH, -1)
            lsx[:, ch * TOT_NI : (ch + 1) * TOT_NI] = v.astype(np.int16)
        out.append(dict(gidx_w=w, nzg=nzg, lsx=lsx))
    return out

'''

import types

P = types.ModuleType("prep_v2_inline")
exec(_PREP_SRC, P.__dict__)

N = P.N
D = 64
NC = P.NC
RPC = P.RPC
NBLK = P.NBLK
CCH = P.CCH
NCHUNK = P.NCHUNK
OUTR = NBLK * 128  # 1280 padded rows per core
GCH = 8  # gather chunk width (grid cols per dma_gather); 1024 idxs/op
NQ = 4  # swdge queues


def _chunk_pieces(layout, grid_off):
    """Per block: list of (goff_cols, width, section) pieces, section 0=A 1=B,
    each at most GCH cols, not crossing the A/B boundary."""
    pieces = []
    for b in range(NBLK):
        lay = layout[b]
        segs = [(0, lay["na"], 0), (lay["na"], lay["nieff"], 1)]
        out = []
        for s0, s1, sec in segs:
            c = s0
            while c < s1:
                w = min(GCH, s1 - c)
                out.append((c, w, sec))
                c += w
        pieces.append(out)
    return pieces


def _build_program(static, pos_cnt, b2f, nidx):
    import concourse.bacc as bacc
    import concourse.mybir as mybir
    import concourse.tile as tile
    from concourse.masks import make_identity

    f32 = mybir.dt.float32
    bf16 = mybir.dt.bfloat16
    i16 = mybir.dt.int16
    add = mybir.AluOpType.add
    mult = mybir.AluOpType.mult
    subtract = mybir.AluOpType.subtract
    AF = mybir.ActivationFunctionType

    layout = static["layout"]
    grid_off = static["grid_off"]
    TOT_NI = static["TOT_NI"]
    TOT_T = static["TOT_T"]
    pieces = _chunk_pieces(layout, grid_off)

    nc = bacc.Bacc(num_swdge_queues=NQ)

    embp = nc.declare_dram_parameter("embed", [N, D], f32, isOutput=False)
    w1p = nc.declare_dram_parameter("w1", [3 * D, D], f32, isOutput=False)
    b1p = nc.declare_dram_parameter("b1r", [1, D], f32, isOutput=False)
    w2p = nc.declare_dram_parameter("w2b", [128, D], f32, isOutput=False)
    gip = nc.declare_dram_parameter("gidx16", [128, TOT_T // 16], i16, isOutput=False)
    lrp = nc.declare_dram_parameter("lrows16", [128, (256 * NBLK) // 16], i16, isOutput=False)
    nzp = nc.declare_dram_parameter("nzg", [128, TOT_NI], f32, isOutput=False)
    lsp = nc.declare_dram_parameter("lsx", [128, NCHUNK * TOT_NI], i16, isOutput=False)
    adjp = nc.declare_dram_parameter("adjp", [OUTR, N], bf16, isOutput=False)
    outp = nc.declare_dram_parameter("out", [OUTR, N], bf16, isOutput=True)

    ct2 = nc.dram_tensor("ct2", [2 * N, D], f32)

    NBLKA = -(-N // 128)  # 79

    with tile.TileContext(nc) as tc:
        with (
            tc.tile_pool(name="const", bufs=1) as cp,
            tc.tile_pool(name="grids", bufs=1) as gp,
            tc.tile_pool(name="stagea", bufs=3) as sp,
            tc.tile_pool(name="work", bufs=2) as wp,
            tc.tile_pool(name="gq", bufs=4) as qp,
            tc.tile_pool(name="psum", bufs=2, space="PSUM") as pp,
        ):
            # ---------------- constants -------------------------------------
            identity = cp.tile([128, 128], f32)
            make_identity(nc, identity[:])
            w1a = cp.tile([D, D], f32)
            nc.sync.dma_start(out=w1a[:], in_=w1p[0:D, :])
            w1b = cp.tile([D, D], f32)
            nc.sync.dma_start(out=w1b[:], in_=w1p[D : 2 * D, :])
            w1c = cp.tile([D, D], f32)
            nc.sync.dma_start(out=w1c[:], in_=w1p[2 * D : 3 * D, :])
            b1t = cp.tile([1, D], f32)
            nc.sync.dma_start(out=b1t[:], in_=b1p[:, :])
            w2t = cp.tile([128, D], f32)
            nc.sync.dma_start(out=w2t[:], in_=w2p[:, :])
            ones = cp.tile([1, 128], f32)
            nc.vector.memset(ones[:], 1.0)
            e5 = cp.tile([D, 1], f32)
            nc.sync.dma_start(
                out=e5[:], in_=embp[nidx : nidx + 1, :].rearrange("o d -> d o")
            )

            # persistent per-core grids
            gidx16 = gp.tile([128, TOT_T // 16], i16)
            nc.sync.dma_start(out=gidx16[:], in_=gip[:, :])
            lrows16 = gp.tile([128, (256 * NBLK) // 16], i16)
            nc.sync.dma_start(out=lrows16[:], in_=lrp[:, :])
            nzg = gp.tile([128, TOT_NI], f32)
            nc.sync.dma_start(out=nzg[:], in_=nzp[:, :])
            lsx = gp.tile([128, NCHUNK * TOT_NI], i16)
            nc.sync.dma_start(out=lsx[:], in_=lsp[:, :])
            sgrid = gp.tile([128, TOT_NI], f32)
            dgrid = gp.tile([128, TOT_NI], bf16)

            # c_vec = embed[nidx] @ W1c + b1 -> [1, D]
            cps = pp.tile([1, D], f32, tag="cps")
            nc.tensor.matmul(cps[:], lhsT=e5[:], rhs=w1c[:], start=True, stop=True)
            crow = cp.tile([1, D], f32)
            nc.vector.tensor_tensor(out=crow[:], in0=cps[:], in1=b1t[:], op=add)

            # ---------------- phase A: tables -> ct2 ------------------------
            for blk in range(NBLKA):
                r0 = blk * 128
                p = min(128, N - r0)
                et = sp.tile([128, D], f32, tag="et")
                nc.sync.dma_start(out=et[:p, :], in_=embp[r0 : r0 + p, :])
                tps = pp.tile([D, 128], f32, tag="tps")
                nc.tensor.transpose(tps[:, :p], et[:p, :], identity[:p, :p])
                tsb = sp.tile([D, 128], f32, tag="tsb")
                nc.scalar.copy(out=tsb[:, :p], in_=tps[:, :p])
                pa_ = pp.tile([128, D], f32, tag="pa")
                nc.tensor.matmul(
                    pa_[:p, :], lhsT=tsb[:, :p], rhs=w1a[:], start=True, stop=False
                )
                nc.tensor.matmul(
                    pa_[:p, :], lhsT=ones[:, :p], rhs=crow[:], start=False, stop=True
                )
                asb = sp.tile([128, D], f32, tag="asb")
                nc.vector.tensor_tensor(
                    out=asb[:p, :], in0=pa_[:p, :], in1=w2t[:p, :], op=mult
                )
                nc.sync.dma_start(out=ct2[N + r0 : N + r0 + p, :], in_=asb[:p, :])
                pb_ = pp.tile([128, D], f32, tag="pb")
                nc.tensor.matmul(
                    pb_[:p, :], lhsT=tsb[:, :p], rhs=w1b[:], start=True, stop=True
                )
                bsb = sp.tile([128, D], f32, tag="bsb")
                nc.vector.tensor_tensor(
                    out=bsb[:p, :], in0=pb_[:p, :], in1=w2t[:p, :], op=mult
                )
                nc.sync.dma_start(out=ct2[r0 : r0 + p, :], in_=bsb[:p, :])

            # ---------------- phase B1: gates per block ---------------------
            for b in range(NBLK):
                lay = layout[b]
                g0 = int(grid_off[b])
                nieff = lay["nieff"]
                # local table vectors for this block: [128, 2, 64]
                lb = wp.tile([128, 2 * D], f32, tag="lb")
                nc.gpsimd.dma_gather(
                    out_ap=lb[:].rearrange("p (s d) -> p s d", d=D),
                    in_ap=ct2[:, :],
                    idxs_ap=lrows16[:, b * 16 : (b + 1) * 16],
                    num_idxs=256,
                    num_idxs_reg=256,
                    elem_size=D,
                    queue_num=b % NQ,
                )
                lb3 = lb[:].rearrange("p (s d) -> p s d", d=D)

                for pi, (coff, w, sec) in enumerate(pieces[b]):
                    g = qp.tile([128, GCH * D], f32, tag="g")
                    g3 = g[:, 0 : w * D].rearrange("p (s d) -> p s d", d=D)
                    f0 = (g0 + coff) * 128
                    nc.gpsimd.dma_gather(
                        out_ap=g3,
                        in_ap=ct2[:, :],
                        idxs_ap=gidx16[:, f0 // 16 : (f0 + w * 128) // 16],
                        num_idxs=w * 128,
                        num_idxs_reg=w * 128,
                        elem_size=D,
                        queue_num=pi % NQ,
                    )
                    nc.vector.tensor_tensor(
                        out=g3,
                        in0=g3,
                        in1=lb3[:, sec : sec + 1, :].to_broadcast([128, w, D]),
                        op=add,
                    )
                    nc.scalar.activation(
                        out=g[:, 0 : w * D], in_=g[:, 0 : w * D], func=AF.Relu
                    )
                    sv = sgrid[:, g0 + coff : g0 + coff + w]
                    if pos_cnt == D:
                        nc.vector.tensor_reduce(
                            out=sv, in_=g3, axis=mybir.AxisListType.X, op=add
                        )
                    elif pos_cnt == 0:
                        nc.vector.tensor_reduce(
                            out=sv, in_=g3, axis=mybir.AxisListType.X, op=add,
                            negate=True,
                        )
                    else:
                        nc.vector.tensor_reduce(
                            out=sv, in_=g3[:, :, :pos_cnt],
                            axis=mybir.AxisListType.X, op=add,
                        )
                        sn = wp.tile([128, GCH], f32, tag="sn")
                        nc.vector.tensor_reduce(
                            out=sn[:, 0:w], in_=g3[:, :, pos_cnt:],
                            axis=mybir.AxisListType.X, op=add,
                        )
                        nc.vector.tensor_tensor(
                            out=sv, in0=sv, in1=sn[:, 0:w], op=subtract
                        )

                # tail in wrap layout over the whole block
                assert nieff <= 128
                nzv = nzg[:, g0 : g0 + nieff]
                om = wp.tile([128, 128], f32, tag="om")
                omv = om[:, 0:nieff]
                nc.vector.tensor_scalar(
                    out=omv, in0=nzv, scalar1=-1.0, scalar2=1.0, op0=mult, op1=add
                )
                ln1 = wp.tile([128, 128], f32, tag="ln1")
                l1v = ln1[:, 0:nieff]
                nc.scalar.activation(out=l1v, in_=nzv, func=AF.Ln)
                ln2 = wp.tile([128, 128], f32, tag="ln2")
                l2v = ln2[:, 0:nieff]
                nc.scalar.activation(out=l2v, in_=omv, func=AF.Ln)
                z = wp.tile([128, 128], f32, tag="z")
                zv = z[:, 0:nieff]
                nc.vector.scalar_tensor_tensor(
                    out=zv, in0=l1v, scalar=b2f, in1=l2v, op0=add, op1=subtract
                )
                nc.vector.tensor_tensor(
                    out=zv, in0=zv, in1=sgrid[:, g0 : g0 + nieff], op=add
                )
                gt = wp.tile([128, 128], f32, tag="gt")
                gtv = gt[:, 0:nieff]
                nc.scalar.activation(out=gtv, in_=zv, func=AF.Sigmoid)
                dv = dgrid[:, g0 : g0 + nieff]
                nc.vector.tensor_scalar_mul(out=dv, in0=gtv, scalar1=0.5)
                # duplicate-cell strip folds
                for dst, src, wd in lay["passes"]:
                    nc.vector.tensor_tensor(
                        out=dgrid[:, g0 + dst : g0 + dst + wd],
                        in0=dgrid[:, g0 + dst : g0 + dst + wd],
                        in1=dgrid[:, g0 + src : g0 + src + wd],
                        op=add,
                    )

            # ---------------- phase B2: dense apply -------------------------
            for b in range(NBLK):
                lay = layout[b]
                g0 = int(grid_off[b])
                nieff = lay["nieff"]
                for ch in range(NCHUNK):
                    msk = wp.tile([128, CCH], bf16, tag="msk")
                    nc.gpsimd.local_scatter(
                        out_ap=msk[:],
                        data_ap=dgrid[:, g0 : g0 + nieff],
                        idxs_ap=lsx[:, ch * TOT_NI + g0 : ch * TOT_NI + g0 + nieff],
                        channels=128,
                        num_elems=CCH,
                        num_idxs=nieff,
                    )
                    adjc = wp.tile([128, CCH], bf16, tag="adjc")
                    nc.sync.dma_start(
                        out=adjc[:],
                        in_=adjp[b * 128 : (b + 1) * 128, ch * CCH : (ch + 1) * CCH],
                    )
                    oc = wp.tile([128, CCH], bf16, tag="oc")
                    nc.vector.tensor_tensor(
                        out=oc[:], in0=msk[:], in1=adjc[:], op=mult
                    )
                    nc.sync.dma_start(
                        out=outp[b * 128 : (b + 1) * 128, ch * CCH : (ch + 1) * CCH],
                        in_=oc[:],
                    )

    nc.compile()
    return nc


def kernel(embed, row, col, adj, noise, W1, b1, W2, b2, node_idx):
    import ml_dtypes
    from concourse.bass_utils import run_bass_kernel_spmd

    embed = np.ascontiguousarray(np.asarray(embed), dtype=np.float32)
    adj = np.asarray(adj)
    W1 = np.ascontiguousarray(np.asarray(W1), dtype=np.float32)
    b1 = np.ascontiguousarray(np.asarray(b1), dtype=np.float32).ravel()
    W2 = np.ascontiguousarray(np.asarray(W2), dtype=np.float32)
    b2f = float(np.asarray(b2, dtype=np.float32).ravel()[0])
    nidx = int(np.asarray(node_idx))

    # hidden permutation: w2 >= 0 first; |w2| folded into tables on device
    w2v = W2.reshape(-1).astype(np.float32)
    order = np.argsort(w2v < 0, kind="stable")
    pos_cnt = int((w2v >= 0).sum())
    W1p = np.ascontiguousarray(W1[:, order])
    b1p = np.ascontiguousarray(b1[order]).reshape(1, D)
    w2b = np.ascontiguousarray(np.tile(np.abs(w2v[order]).reshape(1, D), (128, 1)))

    cores, static = P.prep(row, col, noise)
    dev = P.device_arrays(cores, static)
    TOT_NI = static["TOT_NI"]

    nc = _build_program(static, pos_cnt, b2f, nidx)

    adj_bf = adj.astype(ml_dtypes.bfloat16)
    in_maps = []
    for k in range(NC):
        # local rows idx per block, wrapped: flat[s*128+p]:
        #   s in {0,1}: s=0 -> A-local row = 10000+min(gbase+p, N-1)
        #               s=1 -> B-local row = min(gbase+p, N-1)
        lflat = np.zeros(256 * NBLK, np.int64)
        for b in range(NBLK):
            gbase = k * RPC + b * 128
            rws = np.minimum(gbase + np.arange(128), N - 1)
            lflat[b * 256 : b * 256 + 128] = N + rws
            lflat[b * 256 + 128 : b * 256 + 256] = rws
        lr16 = np.zeros((128, (256 * NBLK) // 16), np.int16)
        s = np.arange((256 * NBLK) // 16)
        for p in range(128):
            lr16[p, :] = lflat[s * 16 + p % 16]

        adjpad = np.zeros((OUTR, N), ml_dtypes.bfloat16)
        sl = adj_bf[k * RPC : (k + 1) * RPC]
        for b in range(NBLK):
            nreal = min(128, RPC - b * 128)
            adjpad[b * 128 : b * 128 + nreal] = sl[b * 128 : b * 128 + nreal]

        in_maps.append(
            dict(
                embed=embed, w1=W1p, b1r=b1p, w2b=w2b,
                gidx16=dev[k]["gidx_w"], lrows16=lr16,
                nzg=dev[k]["nzg"], lsx=dev[k]["lsx"], adjp=adjpad,
            )
        )

    res = run_bass_kernel_spmd(nc, in_maps, list(range(NC)))
    kernel.last_exec_time_ns = res.exec_time_ns
    kernel.last_result = res

    pieces = []
    for k in range(NC):
        o = np.asarray(res.results[k]["out"]).astype(np.float32)
        for b in range(NBLK):
            nreal = min(128, RPC - b * 128)
            pieces.append(o[b * 128 : b * 128 + nreal, :])
    return np.concatenate(pieces, axis=0)


kernel.last_exec_time_ns = None


# revision 3
# speedup vs baseline: 1.5255x; 1.2609x over previous
"""Trainium2 Bass kernel v2 for the GNN ExplainModule (masked adjacency).

Per core (8 cores, 1250 output rows each, 10 blocks of 128 rows):
  Phase A: tables A=(embed@W1a+cvec)|w2|, B=(embed@W1b)|w2| (hidden permuted
           pos-w2-first) -> DRAM CT2 [20000, 64] f32 (rows 0-9999=B, rest=A).
  Phase B1 (per block): SWDGE dma_gather of each token's random-endpoint
           table row into the wrap grid [128, NIeff, 64]; add the dest-row
           local table vector (per-partition broadcast); relu; pos/neg
           free-dim reduce -> per-token logit; sigmoid tail in wrap layout
           -> bf16 gate grid; duplicate-cell strips folded in with plain
           slice adds (pad slots have noise=1e-30 -> gate==0).
  Phase B2 (per block x 5 col-chunks): gpsimd.local_scatter builds the dense
           [128, 2000] bf16 mask chunk; multiply with the bf16 adj chunk;
           DMA to the bf16 output. Host upcasts and reassembles.
"""

import sys

import numpy as np

for _p in ("/opt/trn_rl_repo",):
    if _p not in sys.path:
        sys.path.insert(0, _p)

_PREP_SRC = r'''
"""Host-side routing/packing for the grid-based dense-apply kernel (v2).

Each edge (r, c) yields two tokens (contributions of 0.5*gate_e):
  type-1 -> cell (r, c): local table = A[dest row r], random = B[dest col c]
  type-2 -> cell (c, r): local table = B[dest row c], random = A[dest col r]

Combined random-gather table CT2 rows: 0..9999 = B, 10000..19999 = A.
  token gather idx = dc + 10000*type   (type in {0,1})

Grid: per (core, block) a [128, NIeff] slot grid; partition = dest row % 128;
flat (SWDGE-wrap) order j*128 + p. Columns, in order:
  A-section (A-local tiled add):  [T1 | P0 | Q(j,tq=0,tp) ...]
  B-section (B-local tiled add):  [T2 | P1 | Q(j,tq=1,tp) ...]
T1/T2: plain tokens. P0/P1: duplicate-cell primaries (type 0/1). Q(j,tq,tp):
j-th extra token (type tq) of a dup cell whose primary has type tp; device
adds Q strip into P strip (plain slice add) after gates are computed. Pad
slots get noise=PAD_NOISE so their gate underflows to 0 (no masks needed).
Primaries and T1/T2 slots carry the local_scatter column index; partners and
pads carry -1.
"""

import numpy as np

N = 10000
E = 320000
D = 64
NC = 8
RPC = N // NC  # 1250
BLK = 128
NBLK = -(-RPC // BLK)  # 10
CCH = 2000
NCHUNK = 5
PAD_NOISE = np.float32(1e-30)


def prep(row, col, noise):
    row = np.asarray(row).astype(np.int64).ravel()
    col = np.asarray(col).astype(np.int64).ravel()
    noise = np.asarray(noise).astype(np.float32).ravel()

    dr = np.concatenate([row, col])
    dc = np.concatenate([col, row])
    typ = np.concatenate([np.zeros(E, np.int64), np.ones(E, np.int64)])
    nz = np.concatenate([noise, noise])
    core = dr // RPC

    # ---- pass 1: group tokens, find dup chains, collect static sizes ------
    # cells[k][(bi,p)] = dict(t1=[(dc,ty,nz)...], t2=[...], chains=[[tok...]])
    percore = []
    NI1 = np.zeros(NBLK, np.int64)
    NI2 = np.zeros(NBLK, np.int64)
    WP0 = np.zeros(NBLK, np.int64)
    WP1 = np.zeros(NBLK, np.int64)
    WQ = [{} for _ in range(NBLK)]  # (j, tq, tp) -> width

    for k in range(NC):
        m = core == k
        rl = (dr[m] - k * RPC).astype(np.int64)
        dck = dc[m]
        tyk = typ[m]
        nzk = nz[m]
        bi = rl // BLK
        p = rl % BLK

        cell = rl * N + dck
        order = np.argsort(cell, kind="stable")
        cs = cell[order]
        starts = np.flatnonzero(np.concatenate(([True], cs[1:] != cs[:-1])))
        counts = np.diff(np.concatenate((starts, [len(cs)])))

        rows = {}
        for si, cnt in zip(starts, counts):
            idxs = order[si : si + cnt]
            i0 = idxs[0]
            key = (int(bi[i0]), int(p[i0]))
            r = rows.setdefault(key, dict(t1=[], t2=[], chains=[]))
            toks = [(int(dck[i]), int(tyk[i]), float(nzk[i])) for i in idxs]
            if cnt == 1:
                (r["t1"] if toks[0][1] == 0 else r["t2"]).append(toks[0])
            else:
                r["chains"].append(toks)
        percore.append(rows)

        for (b, _pp), r in rows.items():
            NI1[b] = max(NI1[b], len(r["t1"]))
            NI2[b] = max(NI2[b], len(r["t2"]))
            np0 = sum(1 for ch in r["chains"] if ch[0][1] == 0)
            np1 = len(r["chains"]) - np0
            WP0[b] = max(WP0[b], np0)
            WP1[b] = max(WP1[b], np1)
            qcnt = {}
            i0 = i1 = 0
            for ch in r["chains"]:
                tp = ch[0][1]
                for j, tok in enumerate(ch[1:], start=1):
                    kq = (j, tok[1], tp)
                    qcnt[kq] = qcnt.get(kq, 0) + 1
            for kq, w in qcnt.items():
                WQ[b][kq] = max(WQ[b].get(kq, 0), w)

    # ---- static grid layout per block -------------------------------------
    layout = []  # per block: dict with region offsets
    for b in range(NBLK):
        off = 0
        t1_off = off
        off += int(NI1[b])
        p0_off = off
        off += int(WP0[b])
        qA = []  # (j,tq=0,tp) strips, A-section
        for kq in sorted(kk for kk in WQ[b] if kk[1] == 0):
            qA.append((kq, off, WQ[b][kq]))
            off += WQ[b][kq]
        na = off
        t2_off = off
        off += int(NI2[b])
        p1_off = off
        off += int(WP1[b])
        qB = []
        for kq in sorted(kk for kk in WQ[b] if kk[1] == 1):
            qB.append((kq, off, WQ[b][kq]))
            off += WQ[b][kq]
        nieff = off
        if nieff % 2:
            nieff += 1
        # dup passes: for each Q strip: dst = P strip of tp
        passes = []
        for (j, tq, tp), qoff, w in qA + qB:
            passes.append(((p0_off if tp == 0 else p1_off), qoff, w))
        layout.append(
            dict(
                t1=t1_off, p0=p0_off, na=na, t2=t2_off, p1=p1_off,
                nieff=nieff, passes=passes,
                qA=qA, qB=qB,
            )
        )
    grid_off = np.zeros(NBLK + 1, np.int64)
    for b in range(NBLK):
        grid_off[b + 1] = grid_off[b] + layout[b]["nieff"]
    TOT_NI = int(grid_off[NBLK])
    TOT_T = TOT_NI * 128

    # ---- pass 2: fill per-core slot arrays --------------------------------
    cores = []
    for k in range(NC):
        rows = percore[k]
        gidx = np.zeros((128, TOT_NI), np.int64)  # gather row in CT2
        nzg = np.full((128, TOT_NI), PAD_NOISE, np.float32)
        lscol = np.full((128, TOT_NI), -1, np.int64)  # dest col or -1
        for (b, pp), r in rows.items():
            lay = layout[b]
            g0 = grid_off[b]

            def put(slot, tok, scatter):
                dcv, ty, nzv = tok
                gidx[pp, g0 + slot] = dcv + 10000 * ty
                nzg[pp, g0 + slot] = nzv
                if scatter:
                    lscol[pp, g0 + slot] = dcv

            for i, tok in enumerate(r["t1"]):
                put(lay["t1"] + i, tok, True)
            for i, tok in enumerate(r["t2"]):
                put(lay["t2"] + i, tok, True)
            ip0 = ip1 = 0
            qfill = {}
            for ch in r["chains"]:
                tp = ch[0][1]
                if tp == 0:
                    islot = ip0
                    ip0 += 1
                    put(lay["p0"] + islot, ch[0], True)
                else:
                    islot = ip1
                    ip1 += 1
                    put(lay["p1"] + islot, ch[0], True)
                for j, tok in enumerate(ch[1:], start=1):
                    kq = (j, tok[1], tp)
                    qoff = None
                    for kk, qo, w in lay["qA"] + lay["qB"]:
                        if kk == kq:
                            qoff = qo
                            break
                    # slot within strip must MATCH the primary's islot so the
                    # plain slice add P[:, 0:w] += Q[:, 0:w] lands correctly
                    put(qoff + islot, tok, False)
                    qfill.setdefault((pp, kq), []).append(islot)

        # sanity: within a strip, each (partition, islot) used at most once
        cores.append(dict(gidx=gidx, nzg=nzg, lscol=lscol))

    static = dict(
        layout=layout, grid_off=grid_off, TOT_NI=TOT_NI, TOT_T=TOT_T,
        NI1=NI1, NI2=NI2, WP0=WP0, WP1=WP1,
    )
    return cores, static


def device_arrays(cores, static):
    """Pack per-core arrays into the device input formats."""
    TOT_NI = static["TOT_NI"]
    TOT_T = static["TOT_T"]
    out = []
    for c in cores:
        gidx = c["gidx"]  # [128, TOT_NI] row in CT2
        # flat order f = j*128 + p  ->  wrapped [128, TOT_T//16] int16:
        flat = np.empty(TOT_T, np.int64)
        flat.reshape(TOT_NI, 128)[:] = gidx.T  # flat[j*128+p] = gidx[p, j]
        w = np.empty((128, TOT_T // 16), np.int16)
        s = np.arange(TOT_T // 16)
        for p in range(128):
            w[p, :] = flat[s * 16 + p % 16]
        nzg = c["nzg"].astype(np.float32)
        lsx = np.full((128, NCHUNK * TOT_NI), -1, np.int16)
        for ch in range(NCHUNK):
            sel = (c["lscol"] >= ch * CCH) & (c["lscol"] < (ch + 1) * CCH)
            v = np.where(sel, c["lscol"] - ch * CCH, -1)
            lsx[:, ch * TOT_NI : (ch + 1) * TOT_NI] = v.astype(np.int16)
        out.append(dict(gidx_w=w, nzg=nzg, lsx=lsx))
    return out

'''

import types

P = types.ModuleType("prep_v2_inline")
exec(_PREP_SRC, P.__dict__)

N = P.N
D = 64
NC = P.NC
RPC = P.RPC
NBLK = P.NBLK
CCH = P.CCH
NCHUNK = P.NCHUNK
OUTR = NBLK * 128  # 1280 padded rows per core
GCH = 8  # gather chunk width (grid cols per dma_gather); 1024 idxs/op
NQ = 4  # swdge queues


def _chunk_pieces(layout, grid_off):
    """Per block: list of (goff_cols, width, subsegs) pieces of up to GCH cols;
    subsegs = [(sub_off, sub_w, section)] covering the piece, split at the
    A/B local-table boundary (section 0 = A-local, 1 = B-local)."""
    pieces = []
    for b in range(NBLK):
        lay = layout[b]
        na, nieff = lay["na"], lay["nieff"]
        out = []
        c = 0
        while c < nieff:
            w = min(GCH, nieff - c)
            subs = []
            lo, hi = c, c + w
            if lo < na:
                subs.append((lo, min(hi, na) - lo, 0))
            if hi > na:
                subs.append((max(lo, na), hi - max(lo, na), 1))
            assert sum(sw for _, sw, _ in subs) == w
            out.append((c, w, subs))
            c += w
        assert sum(w for _, w, _ in out) == nieff
        pieces.append(out)
    return pieces


def _build_program(static, pos_cnt, b2f, nidx):
    import concourse.bacc as bacc
    import concourse.mybir as mybir
    import concourse.tile as tile
    from concourse.masks import make_identity

    f32 = mybir.dt.float32
    bf16 = mybir.dt.bfloat16
    i16 = mybir.dt.int16
    add = mybir.AluOpType.add
    mult = mybir.AluOpType.mult
    subtract = mybir.AluOpType.subtract
    AF = mybir.ActivationFunctionType

    layout = static["layout"]
    grid_off = static["grid_off"]
    TOT_NI = static["TOT_NI"]
    TOT_T = static["TOT_T"]
    pieces = _chunk_pieces(layout, grid_off)

    nc = bacc.Bacc(num_swdge_queues=NQ)

    embp = nc.declare_dram_parameter("embed", [N, D], f32, isOutput=False)
    w1p = nc.declare_dram_parameter("w1", [3 * D, D], f32, isOutput=False)
    b1p = nc.declare_dram_parameter("b1r", [1, D], f32, isOutput=False)
    w2p = nc.declare_dram_parameter("w2b", [128, D], f32, isOutput=False)
    gip = nc.declare_dram_parameter("gidx16", [128, TOT_T // 16], i16, isOutput=False)
    lrp = nc.declare_dram_parameter("lrows16", [128, (256 * NBLK) // 16], i16, isOutput=False)
    nzp = nc.declare_dram_parameter("nzg", [128, TOT_NI], f32, isOutput=False)
    lsp = nc.declare_dram_parameter("lsx", [128, NCHUNK * TOT_NI], i16, isOutput=False)
    adjp = nc.declare_dram_parameter("adjp", [OUTR, N], bf16, isOutput=False)
    outp = nc.declare_dram_parameter("out", [OUTR, N], bf16, isOutput=True)

    ct2 = nc.dram_tensor("ct2", [2 * N, D], f32)

    NBLKA = -(-N // 128)  # 79

    with tile.TileContext(nc) as tc:
        with (
            tc.tile_pool(name="const", bufs=1) as cp,
            tc.tile_pool(name="grids", bufs=1) as gp,
            tc.tile_pool(name="stagea", bufs=5) as sp,
            tc.tile_pool(name="work", bufs=2) as wp,
            tc.tile_pool(name="gq", bufs=4) as qp,
            tc.tile_pool(name="psum", bufs=2, space="PSUM") as pp,
        ):
            # ---------------- constants -------------------------------------
            identity = cp.tile([128, 128], f32)
            make_identity(nc, identity[:])
            w1a = cp.tile([D, D], f32)
            nc.sync.dma_start(out=w1a[:], in_=w1p[0:D, :])
            w1b = cp.tile([D, D], f32)
            nc.sync.dma_start(out=w1b[:], in_=w1p[D : 2 * D, :])
            w1c = cp.tile([D, D], f32)
            nc.sync.dma_start(out=w1c[:], in_=w1p[2 * D : 3 * D, :])
            b1t = cp.tile([1, D], f32)
            nc.sync.dma_start(out=b1t[:], in_=b1p[:, :])
            w2t = cp.tile([128, D], f32)
            nc.sync.dma_start(out=w2t[:], in_=w2p[:, :])
            ones = cp.tile([1, 128], f32)
            nc.vector.memset(ones[:], 1.0)
            e5 = cp.tile([D, 1], f32)
            nc.sync.dma_start(
                out=e5[:], in_=embp[nidx : nidx + 1, :].rearrange("o d -> d o")
            )

            # persistent per-core grids
            gidx16 = gp.tile([128, TOT_T // 16], i16)
            nc.sync.dma_start(out=gidx16[:], in_=gip[:, :])
            lrows16 = gp.tile([128, (256 * NBLK) // 16], i16)
            nc.sync.dma_start(out=lrows16[:], in_=lrp[:, :])
            nzg = gp.tile([128, TOT_NI], f32)
            nc.sync.dma_start(out=nzg[:], in_=nzp[:, :])
            lsx = gp.tile([128, NCHUNK * TOT_NI], i16)
            nc.sync.dma_start(out=lsx[:], in_=lsp[:, :])
            sgrid = gp.tile([128, TOT_NI], f32)
            dgrid = gp.tile([128, TOT_NI], bf16)

            # c_vec = embed[nidx] @ W1c + b1 -> [1, D]
            cps = pp.tile([1, D], f32, tag="cps")
            nc.tensor.matmul(cps[:], lhsT=e5[:], rhs=w1c[:], start=True, stop=True)
            crow = cp.tile([1, D], f32)
            nc.vector.tensor_tensor(out=crow[:], in0=cps[:], in1=b1t[:], op=add)

            # ---------------- phase A: tables -> ct2 ------------------------
            for blk in range(NBLKA):
                r0 = blk * 128
                p = min(128, N - r0)
                et = sp.tile([128, D], f32, tag="et")
                nc.sync.dma_start(out=et[:p, :], in_=embp[r0 : r0 + p, :])
                tps = pp.tile([D, 128], f32, tag="tps")
                nc.tensor.transpose(tps[:, :p], et[:p, :], identity[:p, :p])
                tsb = sp.tile([D, 128], f32, tag="tsb")
                nc.scalar.copy(out=tsb[:, :p], in_=tps[:, :p])
                pa_ = pp.tile([128, D], f32, tag="pa")
                nc.tensor.matmul(
                    pa_[:p, :], lhsT=tsb[:, :p], rhs=w1a[:], start=True, stop=False
                )
                nc.tensor.matmul(
                    pa_[:p, :], lhsT=ones[:, :p], rhs=crow[:], start=False, stop=True
                )
                asb = sp.tile([128, D], f32, tag="asb")
                nc.vector.tensor_tensor(
                    out=asb[:p, :], in0=pa_[:p, :], in1=w2t[:p, :], op=mult
                )
                nc.sync.dma_start(out=ct2[N + r0 : N + r0 + p, :], in_=asb[:p, :])
                pb_ = pp.tile([128, D], f32, tag="pb")
                nc.tensor.matmul(
                    pb_[:p, :], lhsT=tsb[:, :p], rhs=w1b[:], start=True, stop=True
                )
                bsb = sp.tile([128, D], f32, tag="bsb")
                nc.vector.tensor_tensor(
                    out=bsb[:p, :], in0=pb_[:p, :], in1=w2t[:p, :], op=mult
                )
                nc.sync.dma_start(out=ct2[r0 : r0 + p, :], in_=bsb[:p, :])

            # ---------------- phase B1: gates per block ---------------------
            for b in range(NBLK):
                lay = layout[b]
                g0 = int(grid_off[b])
                nieff = lay["nieff"]
                # local table vectors for this block: [128, 2, 64]
                lb = wp.tile([128, 2 * D], f32, tag="lb")
                nc.gpsimd.dma_gather(
                    out_ap=lb[:].rearrange("p (s d) -> p s d", d=D),
                    in_ap=ct2[:, :],
                    idxs_ap=lrows16[:, b * 16 : (b + 1) * 16],
                    num_idxs=256,
                    num_idxs_reg=256,
                    elem_size=D,
                    queue_num=b % NQ,
                )
                lb3 = lb[:].rearrange("p (s d) -> p s d", d=D)

                for pi, (coff, w, subs) in enumerate(pieces[b]):
                    g = qp.tile([128, GCH * D], f32, tag="g")
                    g3 = g[:, 0 : w * D].rearrange("p (s d) -> p s d", d=D)
                    f0 = (g0 + coff) * 128
                    nc.gpsimd.dma_gather(
                        out_ap=g3,
                        in_ap=ct2[:, :],
                        idxs_ap=gidx16[:, f0 // 16 : (f0 + w * 128) // 16],
                        num_idxs=w * 128,
                        num_idxs_reg=w * 128,
                        elem_size=D,
                        queue_num=pi % NQ,
                    )
                    for (s0_, sw, sec) in subs:
                        gsub = g3[:, s0_ - coff : s0_ - coff + sw, :]
                        nc.vector.tensor_tensor(
                            out=gsub,
                            in0=gsub,
                            in1=lb3[:, sec : sec + 1, :].to_broadcast(
                                [128, sw, D]
                            ),
                            op=add,
                        )
                    nc.scalar.activation(
                        out=g[:, 0 : w * D], in_=g[:, 0 : w * D], func=AF.Relu
                    )
                    sv = sgrid[:, g0 + coff : g0 + coff + w]
                    if pos_cnt == D:
                        nc.vector.tensor_reduce(
                            out=sv, in_=g3, axis=mybir.AxisListType.X, op=add
                        )
                    elif pos_cnt == 0:
                        nc.vector.tensor_reduce(
                            out=sv, in_=g3, axis=mybir.AxisListType.X, op=add,
                            negate=True,
                        )
                    else:
                        nc.vector.tensor_reduce(
                            out=sv, in_=g3[:, :, :pos_cnt],
                            axis=mybir.AxisListType.X, op=add,
                        )
                        sn = wp.tile([128, GCH], f32, tag="sn")
                        nc.vector.tensor_reduce(
                            out=sn[:, 0:w], in_=g3[:, :, pos_cnt:],
                            axis=mybir.AxisListType.X, op=add,
                        )
                        nc.vector.tensor_tensor(
                            out=sv, in0=sv, in1=sn[:, 0:w], op=subtract
                        )

                # tail in wrap layout over the whole block
                assert nieff <= 128
                nzv = nzg[:, g0 : g0 + nieff]
                om = wp.tile([128, 128], f32, tag="om")
                omv = om[:, 0:nieff]
                nc.vector.tensor_scalar(
                    out=omv, in0=nzv, scalar1=-1.0, scalar2=1.0, op0=mult, op1=add
                )
                ln1 = wp.tile([128, 128], f32, tag="ln1")
                l1v = ln1[:, 0:nieff]
                nc.scalar.activation(out=l1v, in_=nzv, func=AF.Ln)
                ln2 = wp.tile([128, 128], f32, tag="ln2")
                l2v = ln2[:, 0:nieff]
                nc.scalar.activation(out=l2v, in_=omv, func=AF.Ln)
                z = wp.tile([128, 128], f32, tag="z")
                zv = z[:, 0:nieff]
                nc.vector.scalar_tensor_tensor(
                    out=zv, in0=l1v, scalar=b2f, in1=l2v, op0=add, op1=subtract
                )
                nc.vector.tensor_tensor(
                    out=zv, in0=zv, in1=sgrid[:, g0 : g0 + nieff], op=add
                )
                gt = wp.tile([128, 128], f32, tag="gt")
                gtv = gt[:, 0:nieff]
                nc.scalar.activation(out=gtv, in_=zv, func=AF.Sigmoid)
                dv = dgrid[:, g0 : g0 + nieff]
                nc.vector.tensor_scalar_mul(out=dv, in0=gtv, scalar1=0.5)
                # duplicate-cell strip folds
                for dst, src, wd in lay["passes"]:
                    nc.vector.tensor_tensor(
                        out=dgrid[:, g0 + dst : g0 + dst + wd],
                        in0=dgrid[:, g0 + dst : g0 + dst + wd],
                        in1=dgrid[:, g0 + src : g0 + src + wd],
                        op=add,
                    )

            # ---------------- phase B2: dense apply -------------------------
            for b in range(NBLK):
                lay = layout[b]
                g0 = int(grid_off[b])
                nieff = lay["nieff"]
                for ch in range(NCHUNK):
                    msk = wp.tile([128, CCH], bf16, tag="msk")
                    nc.gpsimd.local_scatter(
                        out_ap=msk[:],
                        data_ap=dgrid[:, g0 : g0 + nieff],
                        idxs_ap=lsx[:, ch * TOT_NI + g0 : ch * TOT_NI + g0 + nieff],
                        channels=128,
                        num_elems=CCH,
                        num_idxs=nieff,
                    )
                    adjc = wp.tile([128, CCH], bf16, tag="adjc")
                    nc.sync.dma_start(
                        out=adjc[:],
                        in_=adjp[b * 128 : (b + 1) * 128, ch * CCH : (ch + 1) * CCH],
                    )
                    oc = wp.tile([128, CCH], bf16, tag="oc")
                    nc.vector.tensor_tensor(
                        out=oc[:], in0=msk[:], in1=adjc[:], op=mult
                    )
                    nc.sync.dma_start(
                        out=outp[b * 128 : (b + 1) * 128, ch * CCH : (ch + 1) * CCH],
                        in_=oc[:],
                    )

    nc.compile()
    return nc


def kernel(embed, row, col, adj, noise, W1, b1, W2, b2, node_idx):
    import ml_dtypes
    from concourse.bass_utils import run_bass_kernel_spmd

    embed = np.ascontiguousarray(np.asarray(embed), dtype=np.float32)
    adj = np.asarray(adj)
    W1 = np.ascontiguousarray(np.asarray(W1), dtype=np.float32)
    b1 = np.ascontiguousarray(np.asarray(b1), dtype=np.float32).ravel()
    W2 = np.ascontiguousarray(np.asarray(W2), dtype=np.float32)
    b2f = float(np.asarray(b2, dtype=np.float32).ravel()[0])
    nidx = int(np.asarray(node_idx))

    # hidden permutation: w2 >= 0 first; |w2| folded into tables on device
    w2v = W2.reshape(-1).astype(np.float32)
    order = np.argsort(w2v < 0, kind="stable")
    pos_cnt = int((w2v >= 0).sum())
    W1p = np.ascontiguousarray(W1[:, order])
    b1p = np.ascontiguousarray(b1[order]).reshape(1, D)
    w2b = np.ascontiguousarray(np.tile(np.abs(w2v[order]).reshape(1, D), (128, 1)))

    cores, static = P.prep(row, col, noise)
    dev = P.device_arrays(cores, static)
    TOT_NI = static["TOT_NI"]

    nc = _build_program(static, pos_cnt, b2f, nidx)

    adj_bf = adj.astype(ml_dtypes.bfloat16)
    in_maps = []
    for k in range(NC):
        # local rows idx per block, wrapped: flat[s*128+p]:
        #   s in {0,1}: s=0 -> A-local row = 10000+min(gbase+p, N-1)
        #               s=1 -> B-local row = min(gbase+p, N-1)
        lflat = np.zeros(256 * NBLK, np.int64)
        for b in range(NBLK):
            gbase = k * RPC + b * 128
            rws = np.minimum(gbase + np.arange(128), N - 1)
            lflat[b * 256 : b * 256 + 128] = N + rws
            lflat[b * 256 + 128 : b * 256 + 256] = rws
        lr16 = np.zeros((128, (256 * NBLK) // 16), np.int16)
        s = np.arange((256 * NBLK) // 16)
        for p in range(128):
            lr16[p, :] = lflat[s * 16 + p % 16]

        adjpad = np.zeros((OUTR, N), ml_dtypes.bfloat16)
        sl = adj_bf[k * RPC : (k + 1) * RPC]
        for b in range(NBLK):
            nreal = min(128, RPC - b * 128)
            adjpad[b * 128 : b * 128 + nreal] = sl[b * 128 : b * 128 + nreal]

        in_maps.append(
            dict(
                embed=embed, w1=W1p, b1r=b1p, w2b=w2b,
                gidx16=dev[k]["gidx_w"], lrows16=lr16,
                nzg=dev[k]["nzg"], lsx=dev[k]["lsx"], adjp=adjpad,
            )
        )

    res = run_bass_kernel_spmd(nc, in_maps, list(range(NC)))
    kernel.last_exec_time_ns = res.exec_time_ns
    kernel.last_result = res

    pieces = []
    for k in range(NC):
        o = np.asarray(res.results[k]["out"]).astype(np.float32)
        for b in range(NBLK):
            nreal = min(128, RPC - b * 128)
            pieces.append(o[b * 128 : b * 128 + nreal, :])
    return np.concatenate(pieces, axis=0)


kernel.last_exec_time_ns = None


# revision 4
# speedup vs baseline: 1.5548x; 1.0192x over previous
"""Trainium2 Bass kernel v2 for the GNN ExplainModule (masked adjacency).

Per core (8 cores, 1250 output rows each, 10 blocks of 128 rows):
  Phase A: tables A=(embed@W1a+cvec)|w2|, B=(embed@W1b)|w2| (hidden permuted
           pos-w2-first) -> DRAM CT2 [20000, 64] f32 (rows 0-9999=B, rest=A).
  Phase B1 (per block): SWDGE dma_gather of each token's random-endpoint
           table row into the wrap grid [128, NIeff, 64]; add the dest-row
           local table vector (per-partition broadcast); relu; pos/neg
           free-dim reduce -> per-token logit; sigmoid tail in wrap layout
           -> bf16 gate grid; duplicate-cell strips folded in with plain
           slice adds (pad slots have noise=1e-30 -> gate==0).
  Phase B2 (per block x 5 col-chunks): gpsimd.local_scatter builds the dense
           [128, 2000] bf16 mask chunk; multiply with the bf16 adj chunk;
           DMA to the bf16 output. Host upcasts and reassembles.
"""

import sys

import numpy as np

for _p in ("/opt/trn_rl_repo",):
    if _p not in sys.path:
        sys.path.insert(0, _p)

_PREP_SRC = r'''
"""Host-side routing/packing for the grid-based dense-apply kernel (v2).

Each edge (r, c) yields two tokens (contributions of 0.5*gate_e):
  type-1 -> cell (r, c): local table = A[dest row r], random = B[dest col c]
  type-2 -> cell (c, r): local table = B[dest row c], random = A[dest col r]

Combined random-gather table CT2 rows: 0..9999 = B, 10000..19999 = A.
  token gather idx = dc + 10000*type   (type in {0,1})

Grid: per (core, block) a [128, NIeff] slot grid; partition = dest row % 128;
flat (SWDGE-wrap) order j*128 + p. Columns, in order:
  A-section (A-local tiled add):  [T1 | P0 | Q(j,tq=0,tp) ...]
  B-section (B-local tiled add):  [T2 | P1 | Q(j,tq=1,tp) ...]
T1/T2: plain tokens. P0/P1: duplicate-cell primaries (type 0/1). Q(j,tq,tp):
j-th extra token (type tq) of a dup cell whose primary has type tp; device
adds Q strip into P strip (plain slice add) after gates are computed. Pad
slots get noise=PAD_NOISE so their gate underflows to 0 (no masks needed).
Primaries and T1/T2 slots carry the local_scatter column index; partners and
pads carry -1.
"""

import numpy as np

N = 10000
E = 320000
D = 64
NC = 8
RPC = N // NC  # 1250
BLK = 128
NBLK = -(-RPC // BLK)  # 10
CCH = 2000
NCHUNK = 5
PAD_NOISE = np.float32(1e-30)


def prep(row, col, noise):
    row = np.asarray(row).astype(np.int64).ravel()
    col = np.asarray(col).astype(np.int64).ravel()
    noise = np.asarray(noise).astype(np.float32).ravel()

    dr = np.concatenate([row, col])
    dc = np.concatenate([col, row])
    typ = np.concatenate([np.zeros(E, np.int64), np.ones(E, np.int64)])
    nz = np.concatenate([noise, noise])
    core = dr // RPC

    # ---- pass 1: group tokens, find dup chains, collect static sizes ------
    # cells[k][(bi,p)] = dict(t1=[(dc,ty,nz)...], t2=[...], chains=[[tok...]])
    percore = []
    NI1 = np.zeros(NBLK, np.int64)
    NI2 = np.zeros(NBLK, np.int64)
    WP0 = np.zeros(NBLK, np.int64)
    WP1 = np.zeros(NBLK, np.int64)
    WQ = [{} for _ in range(NBLK)]  # (j, tq, tp) -> width

    for k in range(NC):
        m = core == k
        rl = (dr[m] - k * RPC).astype(np.int64)
        dck = dc[m]
        tyk = typ[m]
        nzk = nz[m]
        bi = rl // BLK
        p = rl % BLK

        cell = rl * N + dck
        order = np.argsort(cell, kind="stable")
        cs = cell[order]
        starts = np.flatnonzero(np.concatenate(([True], cs[1:] != cs[:-1])))
        counts = np.diff(np.concatenate((starts, [len(cs)])))

        rows = {}
        for si, cnt in zip(starts, counts):
            idxs = order[si : si + cnt]
            i0 = idxs[0]
            key = (int(bi[i0]), int(p[i0]))
            r = rows.setdefault(key, dict(t1=[], t2=[], chains=[]))
            toks = [(int(dck[i]), int(tyk[i]), float(nzk[i])) for i in idxs]
            if cnt == 1:
                (r["t1"] if toks[0][1] == 0 else r["t2"]).append(toks[0])
            else:
                r["chains"].append(toks)
        percore.append(rows)

        for (b, _pp), r in rows.items():
            NI1[b] = max(NI1[b], len(r["t1"]))
            NI2[b] = max(NI2[b], len(r["t2"]))
            np0 = sum(1 for ch in r["chains"] if ch[0][1] == 0)
            np1 = len(r["chains"]) - np0
            WP0[b] = max(WP0[b], np0)
            WP1[b] = max(WP1[b], np1)
            qcnt = {}
            i0 = i1 = 0
            for ch in r["chains"]:
                tp = ch[0][1]
                for j, tok in enumerate(ch[1:], start=1):
                    kq = (j, tok[1], tp)
                    qcnt[kq] = qcnt.get(kq, 0) + 1
            for kq, w in qcnt.items():
                WQ[b][kq] = max(WQ[b].get(kq, 0), w)

    # ---- static grid layout per block -------------------------------------
    layout = []  # per block: dict with region offsets
    for b in range(NBLK):
        off = 0
        t1_off = off
        off += int(NI1[b])
        p0_off = off
        off += int(WP0[b])
        qA = []  # (j,tq=0,tp) strips, A-section
        for kq in sorted(kk for kk in WQ[b] if kk[1] == 0):
            qA.append((kq, off, WQ[b][kq]))
            off += WQ[b][kq]
        na = off
        t2_off = off
        off += int(NI2[b])
        p1_off = off
        off += int(WP1[b])
        qB = []
        for kq in sorted(kk for kk in WQ[b] if kk[1] == 1):
            qB.append((kq, off, WQ[b][kq]))
            off += WQ[b][kq]
        nieff = off
        if nieff % 2:
            nieff += 1
        # dup passes: for each Q strip: dst = P strip of tp
        passes = []
        for (j, tq, tp), qoff, w in qA + qB:
            passes.append(((p0_off if tp == 0 else p1_off), qoff, w))
        layout.append(
            dict(
                t1=t1_off, p0=p0_off, na=na, t2=t2_off, p1=p1_off,
                nieff=nieff, passes=passes,
                qA=qA, qB=qB,
            )
        )
    grid_off = np.zeros(NBLK + 1, np.int64)
    for b in range(NBLK):
        grid_off[b + 1] = grid_off[b] + layout[b]["nieff"]
    TOT_NI = int(grid_off[NBLK])
    TOT_T = TOT_NI * 128

    # ---- pass 2: fill per-core slot arrays --------------------------------
    cores = []
    for k in range(NC):
        rows = percore[k]
        gidx = np.zeros((128, TOT_NI), np.int64)  # gather row in CT2
        nzg = np.full((128, TOT_NI), PAD_NOISE, np.float32)
        lscol = np.full((128, TOT_NI), -1, np.int64)  # dest col or -1
        for (b, pp), r in rows.items():
            lay = layout[b]
            g0 = grid_off[b]

            def put(slot, tok, scatter):
                dcv, ty, nzv = tok
                gidx[pp, g0 + slot] = dcv + 10000 * ty
                nzg[pp, g0 + slot] = nzv
                if scatter:
                    lscol[pp, g0 + slot] = dcv

            for i, tok in enumerate(r["t1"]):
                put(lay["t1"] + i, tok, True)
            for i, tok in enumerate(r["t2"]):
                put(lay["t2"] + i, tok, True)
            ip0 = ip1 = 0
            qfill = {}
            for ch in r["chains"]:
                tp = ch[0][1]
                if tp == 0:
                    islot = ip0
                    ip0 += 1
                    put(lay["p0"] + islot, ch[0], True)
                else:
                    islot = ip1
                    ip1 += 1
                    put(lay["p1"] + islot, ch[0], True)
                for j, tok in enumerate(ch[1:], start=1):
                    kq = (j, tok[1], tp)
                    qoff = None
                    for kk, qo, w in lay["qA"] + lay["qB"]:
                        if kk == kq:
                            qoff = qo
                            break
                    # slot within strip must MATCH the primary's islot so the
                    # plain slice add P[:, 0:w] += Q[:, 0:w] lands correctly
                    put(qoff + islot, tok, False)
                    qfill.setdefault((pp, kq), []).append(islot)

        # sanity: within a strip, each (partition, islot) used at most once
        cores.append(dict(gidx=gidx, nzg=nzg, lscol=lscol))

    static = dict(
        layout=layout, grid_off=grid_off, TOT_NI=TOT_NI, TOT_T=TOT_T,
        NI1=NI1, NI2=NI2, WP0=WP0, WP1=WP1,
    )
    return cores, static


def device_arrays(cores, static):
    """Pack per-core arrays into the device input formats."""
    TOT_NI = static["TOT_NI"]
    TOT_T = static["TOT_T"]
    out = []
    for c in cores:
        gidx = c["gidx"]  # [128, TOT_NI] row in CT2
        # flat order f = j*128 + p  ->  wrapped [128, TOT_T//16] int16:
        flat = np.empty(TOT_T, np.int64)
        flat.reshape(TOT_NI, 128)[:] = gidx.T  # flat[j*128+p] = gidx[p, j]
        w = np.empty((128, TOT_T // 16), np.int16)
        s = np.arange(TOT_T // 16)
        for p in range(128):
            w[p, :] = flat[s * 16 + p % 16]
        nzg = c["nzg"].astype(np.float32)
        lsx = np.full((128, NCHUNK * TOT_NI), -1, np.int16)
        for ch in range(NCHUNK):
            sel = (c["lscol"] >= ch * CCH) & (c["lscol"] < (ch + 1) * CCH)
            v = np.where(sel, c["lscol"] - ch * CCH, -1)
            lsx[:, ch * TOT_NI : (ch + 1) * TOT_NI] = v.astype(np.int16)
        out.append(dict(gidx_w=w, nzg=nzg, lsx=lsx))
    return out

'''

import types

P = types.ModuleType("prep_v2_inline")
exec(_PREP_SRC, P.__dict__)

N = P.N
D = 64
NC = P.NC
RPC = P.RPC
NBLK = P.NBLK
CCH = P.CCH
NCHUNK = P.NCHUNK
OUTR = NBLK * 128  # 1280 padded rows per core
GCH = 8  # gather chunk width (grid cols per dma_gather); 1024 idxs/op
NQ = 4  # swdge queues


def _chunk_pieces(layout, grid_off):
    """Per block: list of (goff_cols, width, subsegs) pieces of up to GCH cols;
    subsegs = [(sub_off, sub_w, section)] covering the piece, split at the
    A/B local-table boundary (section 0 = A-local, 1 = B-local)."""
    pieces = []
    for b in range(NBLK):
        lay = layout[b]
        na, nieff = lay["na"], lay["nieff"]
        out = []
        c = 0
        while c < nieff:
            w = min(GCH, nieff - c)
            subs = []
            lo, hi = c, c + w
            if lo < na:
                subs.append((lo, min(hi, na) - lo, 0))
            if hi > na:
                subs.append((max(lo, na), hi - max(lo, na), 1))
            assert sum(sw for _, sw, _ in subs) == w
            out.append((c, w, subs))
            c += w
        assert sum(w for _, w, _ in out) == nieff
        pieces.append(out)
    return pieces


def _build_program(static, pos_cnt, b2f, nidx):
    import concourse.bacc as bacc
    import concourse.mybir as mybir
    import concourse.tile as tile
    from concourse.masks import make_identity

    f32 = mybir.dt.float32
    bf16 = mybir.dt.bfloat16
    i16 = mybir.dt.int16
    add = mybir.AluOpType.add
    mult = mybir.AluOpType.mult
    subtract = mybir.AluOpType.subtract
    AF = mybir.ActivationFunctionType

    layout = static["layout"]
    grid_off = static["grid_off"]
    TOT_NI = static["TOT_NI"]
    TOT_T = static["TOT_T"]
    pieces = _chunk_pieces(layout, grid_off)

    nc = bacc.Bacc(num_swdge_queues=NQ)

    embp = nc.declare_dram_parameter("embed", [N, D], f32, isOutput=False)
    w1p = nc.declare_dram_parameter("w1", [3 * D, D], f32, isOutput=False)
    b1p = nc.declare_dram_parameter("b1r", [1, D], f32, isOutput=False)
    w2p = nc.declare_dram_parameter("w2b", [128, D], f32, isOutput=False)
    gip = nc.declare_dram_parameter("gidx16", [128, TOT_T // 16], i16, isOutput=False)
    lrp = nc.declare_dram_parameter("lrows16", [128, (256 * NBLK) // 16], i16, isOutput=False)
    nzp = nc.declare_dram_parameter("nzg", [128, TOT_NI], f32, isOutput=False)
    lsp = nc.declare_dram_parameter("lsx", [128, NCHUNK * TOT_NI], i16, isOutput=False)
    adjp = nc.declare_dram_parameter("adjp", [OUTR, N], bf16, isOutput=False)
    outp = nc.declare_dram_parameter("out", [OUTR, N], bf16, isOutput=True)

    ct2 = nc.dram_tensor("ct2", [2 * N, D], f32)

    NBLKA = -(-N // 128)  # 79

    with tile.TileContext(nc) as tc:
        with (
            tc.tile_pool(name="const", bufs=1) as cp,
            tc.tile_pool(name="grids", bufs=1) as gp,
            tc.tile_pool(name="stagea", bufs=5) as sp,
            tc.tile_pool(name="work", bufs=2) as wp,
            tc.tile_pool(name="gq", bufs=8) as qp,
            tc.tile_pool(name="psum", bufs=2, space="PSUM") as pp,
        ):
            # ---------------- constants -------------------------------------
            identity = cp.tile([128, 128], f32)
            make_identity(nc, identity[:])
            w1a = cp.tile([D, D], f32)
            nc.sync.dma_start(out=w1a[:], in_=w1p[0:D, :])
            w1b = cp.tile([D, D], f32)
            nc.sync.dma_start(out=w1b[:], in_=w1p[D : 2 * D, :])
            w1c = cp.tile([D, D], f32)
            nc.sync.dma_start(out=w1c[:], in_=w1p[2 * D : 3 * D, :])
            b1t = cp.tile([1, D], f32)
            nc.sync.dma_start(out=b1t[:], in_=b1p[:, :])
            w2t = cp.tile([128, D], f32)
            nc.sync.dma_start(out=w2t[:], in_=w2p[:, :])
            ones = cp.tile([1, 128], f32)
            nc.vector.memset(ones[:], 1.0)
            e5 = cp.tile([D, 1], f32)
            nc.sync.dma_start(
                out=e5[:], in_=embp[nidx : nidx + 1, :].rearrange("o d -> d o")
            )

            # persistent per-core grids
            gidx16 = gp.tile([128, TOT_T // 16], i16)
            nc.sync.dma_start(out=gidx16[:], in_=gip[:, :])
            lrows16 = gp.tile([128, (256 * NBLK) // 16], i16)
            nc.sync.dma_start(out=lrows16[:], in_=lrp[:, :])
            nzg = gp.tile([128, TOT_NI], f32)
            nc.sync.dma_start(out=nzg[:], in_=nzp[:, :])
            lsx = gp.tile([128, NCHUNK * TOT_NI], i16)
            nc.sync.dma_start(out=lsx[:], in_=lsp[:, :])
            sgrid = gp.tile([128, TOT_NI], f32)
            dgrid = gp.tile([128, TOT_NI], bf16)

            # c_vec = embed[nidx] @ W1c + b1 -> [1, D]
            cps = pp.tile([1, D], f32, tag="cps")
            nc.tensor.matmul(cps[:], lhsT=e5[:], rhs=w1c[:], start=True, stop=True)
            crow = cp.tile([1, D], f32)
            nc.vector.tensor_tensor(out=crow[:], in0=cps[:], in1=b1t[:], op=add)

            # ---------------- phase A: tables -> ct2 ------------------------
            for blk in range(NBLKA):
                r0 = blk * 128
                p = min(128, N - r0)
                et = sp.tile([128, D], f32, tag="et")
                nc.sync.dma_start(out=et[:p, :], in_=embp[r0 : r0 + p, :])
                tps = pp.tile([D, 128], f32, tag="tps")
                nc.tensor.transpose(tps[:, :p], et[:p, :], identity[:p, :p])
                tsb = sp.tile([D, 128], f32, tag="tsb")
                nc.scalar.copy(out=tsb[:, :p], in_=tps[:, :p])
                pa_ = pp.tile([128, D], f32, tag="pa")
                nc.tensor.matmul(
                    pa_[:p, :], lhsT=tsb[:, :p], rhs=w1a[:], start=True, stop=False
                )
                nc.tensor.matmul(
                    pa_[:p, :], lhsT=ones[:, :p], rhs=crow[:], start=False, stop=True
                )
                asb = sp.tile([128, D], f32, tag="asb")
                nc.vector.tensor_tensor(
                    out=asb[:p, :], in0=pa_[:p, :], in1=w2t[:p, :], op=mult
                )
                nc.sync.dma_start(out=ct2[N + r0 : N + r0 + p, :], in_=asb[:p, :])
                pb_ = pp.tile([128, D], f32, tag="pb")
                nc.tensor.matmul(
                    pb_[:p, :], lhsT=tsb[:, :p], rhs=w1b[:], start=True, stop=True
                )
                bsb = sp.tile([128, D], f32, tag="bsb")
                nc.vector.tensor_tensor(
                    out=bsb[:p, :], in0=pb_[:p, :], in1=w2t[:p, :], op=mult
                )
                nc.sync.dma_start(out=ct2[r0 : r0 + p, :], in_=bsb[:p, :])

            # ---------------- phase B1: gates per block ---------------------
            for b in range(NBLK):
                lay = layout[b]
                g0 = int(grid_off[b])
                nieff = lay["nieff"]
                # local table vectors for this block: [128, 2, 64]
                lb = wp.tile([128, 2 * D], f32, tag="lb")
                nc.gpsimd.dma_gather(
                    out_ap=lb[:].rearrange("p (s d) -> p s d", d=D),
                    in_ap=ct2[:, :],
                    idxs_ap=lrows16[:, b * 16 : (b + 1) * 16],
                    num_idxs=256,
                    num_idxs_reg=256,
                    elem_size=D,
                    queue_num=b % NQ,
                )
                lb3 = lb[:].rearrange("p (s d) -> p s d", d=D)

                for pi, (coff, w, subs) in enumerate(pieces[b]):
                    g = qp.tile([128, GCH * D], f32, tag="g")
                    g3 = g[:, 0 : w * D].rearrange("p (s d) -> p s d", d=D)
                    f0 = (g0 + coff) * 128
                    nc.gpsimd.dma_gather(
                        out_ap=g3,
                        in_ap=ct2[:, :],
                        idxs_ap=gidx16[:, f0 // 16 : (f0 + w * 128) // 16],
                        num_idxs=w * 128,
                        num_idxs_reg=w * 128,
                        elem_size=D,
                        queue_num=pi % NQ,
                    )
                    for (s0_, sw, sec) in subs:
                        gsub = g3[:, s0_ - coff : s0_ - coff + sw, :]
                        nc.vector.tensor_tensor(
                            out=gsub,
                            in0=gsub,
                            in1=lb3[:, sec : sec + 1, :].to_broadcast(
                                [128, sw, D]
                            ),
                            op=add,
                        )
                    nc.scalar.activation(
                        out=g[:, 0 : w * D], in_=g[:, 0 : w * D], func=AF.Relu
                    )
                    sv = sgrid[:, g0 + coff : g0 + coff + w]
                    if pos_cnt == D:
                        nc.vector.tensor_reduce(
                            out=sv, in_=g3, axis=mybir.AxisListType.X, op=add
                        )
                    elif pos_cnt == 0:
                        nc.vector.tensor_reduce(
                            out=sv, in_=g3, axis=mybir.AxisListType.X, op=add,
                            negate=True,
                        )
                    else:
                        nc.vector.tensor_reduce(
                            out=sv, in_=g3[:, :, :pos_cnt],
                            axis=mybir.AxisListType.X, op=add,
                        )
                        sn = wp.tile([128, GCH], f32, tag="sn")
                        nc.vector.tensor_reduce(
                            out=sn[:, 0:w], in_=g3[:, :, pos_cnt:],
                            axis=mybir.AxisListType.X, op=add,
                        )
                        nc.vector.tensor_tensor(
                            out=sv, in0=sv, in1=sn[:, 0:w], op=subtract
                        )

                # tail in wrap layout over the whole block
                assert nieff <= 128
                nzv = nzg[:, g0 : g0 + nieff]
                om = wp.tile([128, 128], f32, tag="om")
                omv = om[:, 0:nieff]
                nc.vector.tensor_scalar(
                    out=omv, in0=nzv, scalar1=-1.0, scalar2=1.0, op0=mult, op1=add
                )
                ln1 = wp.tile([128, 128], f32, tag="ln1")
                l1v = ln1[:, 0:nieff]
                nc.scalar.activation(out=l1v, in_=nzv, func=AF.Ln)
                ln2 = wp.tile([128, 128], f32, tag="ln2")
                l2v = ln2[:, 0:nieff]
                nc.scalar.activation(out=l2v, in_=omv, func=AF.Ln)
                z = wp.tile([128, 128], f32, tag="z")
                zv = z[:, 0:nieff]
                nc.vector.scalar_tensor_tensor(
                    out=zv, in0=l1v, scalar=b2f, in1=l2v, op0=add, op1=subtract
                )
                nc.vector.tensor_tensor(
                    out=zv, in0=zv, in1=sgrid[:, g0 : g0 + nieff], op=add
                )
                gt = wp.tile([128, 128], f32, tag="gt")
                gtv = gt[:, 0:nieff]
                nc.scalar.activation(out=gtv, in_=zv, func=AF.Sigmoid)
                dv = dgrid[:, g0 : g0 + nieff]
                nc.vector.tensor_scalar_mul(out=dv, in0=gtv, scalar1=0.5)
                # duplicate-cell strip folds
                for dst, src, wd in lay["passes"]:
                    nc.vector.tensor_tensor(
                        out=dgrid[:, g0 + dst : g0 + dst + wd],
                        in0=dgrid[:, g0 + dst : g0 + dst + wd],
                        in1=dgrid[:, g0 + src : g0 + src + wd],
                        op=add,
                    )

            # ---------------- phase B2: dense apply -------------------------
            for b in range(NBLK):
                lay = layout[b]
                g0 = int(grid_off[b])
                nieff = lay["nieff"]
                for ch in range(NCHUNK):
                    msk = wp.tile([128, CCH], bf16, tag="msk")
                    nc.gpsimd.local_scatter(
                        out_ap=msk[:],
                        data_ap=dgrid[:, g0 : g0 + nieff],
                        idxs_ap=lsx[:, ch * TOT_NI + g0 : ch * TOT_NI + g0 + nieff],
                        channels=128,
                        num_elems=CCH,
                        num_idxs=nieff,
                    )
                    adjc = wp.tile([128, CCH], bf16, tag="adjc")
                    nc.sync.dma_start(
                        out=adjc[:],
                        in_=adjp[b * 128 : (b + 1) * 128, ch * CCH : (ch + 1) * CCH],
                    )
                    oc = wp.tile([128, CCH], bf16, tag="oc")
                    nc.vector.tensor_tensor(
                        out=oc[:], in0=msk[:], in1=adjc[:], op=mult
                    )
                    nc.sync.dma_start(
                        out=outp[b * 128 : (b + 1) * 128, ch * CCH : (ch + 1) * CCH],
                        in_=oc[:],
                    )

    nc.compile()
    return nc


def kernel(embed, row, col, adj, noise, W1, b1, W2, b2, node_idx):
    import ml_dtypes
    from concourse.bass_utils import run_bass_kernel_spmd

    embed = np.ascontiguousarray(np.asarray(embed), dtype=np.float32)
    adj = np.asarray(adj)
    W1 = np.ascontiguousarray(np.asarray(W1), dtype=np.float32)
    b1 = np.ascontiguousarray(np.asarray(b1), dtype=np.float32).ravel()
    W2 = np.ascontiguousarray(np.asarray(W2), dtype=np.float32)
    b2f = float(np.asarray(b2, dtype=np.float32).ravel()[0])
    nidx = int(np.asarray(node_idx))

    # hidden permutation: w2 >= 0 first; |w2| folded into tables on device
    w2v = W2.reshape(-1).astype(np.float32)
    order = np.argsort(w2v < 0, kind="stable")
    pos_cnt = int((w2v >= 0).sum())
    W1p = np.ascontiguousarray(W1[:, order])
    b1p = np.ascontiguousarray(b1[order]).reshape(1, D)
    w2b = np.ascontiguousarray(np.tile(np.abs(w2v[order]).reshape(1, D), (128, 1)))

    cores, static = P.prep(row, col, noise)
    dev = P.device_arrays(cores, static)
    TOT_NI = static["TOT_NI"]

    nc = _build_program(static, pos_cnt, b2f, nidx)

    adj_bf = adj.astype(ml_dtypes.bfloat16)
    in_maps = []
    for k in range(NC):
        # local rows idx per block, wrapped: flat[s*128+p]:
        #   s in {0,1}: s=0 -> A-local row = 10000+min(gbase+p, N-1)
        #               s=1 -> B-local row = min(gbase+p, N-1)
        lflat = np.zeros(256 * NBLK, np.int64)
        for b in range(NBLK):
            gbase = k * RPC + b * 128
            rws = np.minimum(gbase + np.arange(128), N - 1)
            lflat[b * 256 : b * 256 + 128] = N + rws
            lflat[b * 256 + 128 : b * 256 + 256] = rws
        lr16 = np.zeros((128, (256 * NBLK) // 16), np.int16)
        s = np.arange((256 * NBLK) // 16)
        for p in range(128):
            lr16[p, :] = lflat[s * 16 + p % 16]

        adjpad = np.zeros((OUTR, N), ml_dtypes.bfloat16)
        sl = adj_bf[k * RPC : (k + 1) * RPC]
        for b in range(NBLK):
            nreal = min(128, RPC - b * 128)
            adjpad[b * 128 : b * 128 + nreal] = sl[b * 128 : b * 128 + nreal]

        in_maps.append(
            dict(
                embed=embed, w1=W1p, b1r=b1p, w2b=w2b,
                gidx16=dev[k]["gidx_w"], lrows16=lr16,
                nzg=dev[k]["nzg"], lsx=dev[k]["lsx"], adjp=adjpad,
            )
        )

    res = run_bass_kernel_spmd(nc, in_maps, list(range(NC)))
    kernel.last_exec_time_ns = res.exec_time_ns
    kernel.last_result = res

    pieces = []
    for k in range(NC):
        o = np.asarray(res.results[k]["out"]).astype(np.float32)
        for b in range(NBLK):
            nreal = min(128, RPC - b * 128)
            pieces.append(o[b * 128 : b * 128 + nreal, :])
    return np.concatenate(pieces, axis=0)


kernel.last_exec_time_ns = None
